# revision 3
# baseline (speedup 1.0000x reference)
"""Deterministic radius-graph KNN (N=16384, K=32, cutoff 5.0) on 8 trn2 NeuronCores.

Strategy (data-parallel over query atoms, z-slab sharded):
  Host: z-sort atoms; per 128-query tile build a 3584-wide z-window of candidates.
  Phase A (device): -d2 approx via PE fp32 matmul (K=5 fused: 2*dot - sq_i - sq_j),
    hierarchical top-56 selection per query (14 strided chunks x cap-16 via
    max8/max_index/match_replace, then stage-2 extraction).
  Host: decode candidate indices, gather candidate fields.
  Phase B (device): bit-exact XLA:CPU d2/dd/vec for the 48 best candidates per
    query, via Dekker-split + TwoSum fp32 emulation of the CPU's fma-chain dot.
  Host: final sort of 48 by (d2, j), cutoff mask, self-pair padding, assembly.

The reference's top-k ordering is knife-edge sensitive to d2 rounding, so phase B
reproduces XLA:CPU arithmetic bit-for-bit; the final output matches the oracle
bitwise on uniform inputs.
"""
import numpy as np

import concourse.bacc as bacc
import concourse.mybir as mybir
import concourse.tile as tile
from concourse.bass_utils import run_bass_kernel_spmd

# ---------------- constants (hardcoded for the given problem) ----------------
N = 16384
K = 32
CUTOFF2 = np.float32(25.0)
NCORES = 8
RPC = N // NCORES          # queries per core
TPC = RPC // 128           # row-tiles per core
NTILES = N // 128
W = 3584                   # candidate window (= 7*512 = 14*256)
S = 14                     # strided chunks per window
CAP = 16                   # survivors per chunk
NCAND = S * CAP            # 224
NSEL = 56                  # stage-2 extracted (incl. self)
NKEEP = 48                 # candidates refined in phase B
SENT = -3.0e38
FREE = TPC * NKEEP
NF = 20

F32 = mybir.dt.float32
U32 = mybir.dt.uint32
OP = mybir.AluOpType

_CACHE = {}


# ---------------- device programs ----------------
def _build_phase_a():
    nc = bacc.Bacc("TRN2", target_bir_lowering=False, debug=False,
                   enable_asserts=True, num_devices=NCORES)
    ab = nc.dram_tensor("ab", [TPC, 5, 128 + W], F32, kind="ExternalInput")
    q56 = nc.dram_tensor("q56", [TPC, 128, NSEL], U32, kind="ExternalOutput")
    p224 = nc.dram_tensor("p224", [TPC, 128, NCAND], U32, kind="ExternalOutput")

    with tile.TileContext(nc) as tc:
        with tc.tile_pool(name="inp", bufs=3) as inp, \
             tc.tile_pool(name="big", bufs=2) as big, \
             tc.tile_pool(name="cand", bufs=2) as cp, \
             tc.tile_pool(name="out", bufs=2) as op_, \
             tc.tile_pool(name="psum", bufs=4, space="PSUM") as pp:
            for t in range(TPC):
                tab = inp.tile([5, 128 + W], F32, tag="tab", name="tab")
                nc.sync.dma_start(out=tab, in_=ab[t])
                neg = big.tile([128, W], F32, tag="neg", name="neg")
                for b in range(W // 512):
                    ps = pp.tile([128, 512], F32, tag="ps", name="ps")
                    nc.tensor.matmul(ps, tab[:, :128],
                                     tab[:, 128 + b * 512:128 + (b + 1) * 512],
                                     start=True, stop=True)
                    nc.scalar.activation(neg[:, b * 512:(b + 1) * 512], ps,
                                         mybir.ActivationFunctionType.Copy)
                cand = cp.tile([128, NCAND], F32, tag="cand", name="cand")
                candp = op_.tile([128, NCAND], U32, tag="candp", name="candp")
                for c in range(S):
                    view = neg[:, c:W:S]
                    v0 = cand[:, c * CAP:c * CAP + 8]
                    nc.vector.max(out=v0, in_=view)
                    nc.vector.max_index(candp[:, c * CAP:c * CAP + 8], v0, view)
                    nc.vector.match_replace(out=view, in_to_replace=v0,
                                            in_values=view, imm_value=SENT)
                    v1 = cand[:, c * CAP + 8:c * CAP + 16]
                    nc.vector.max(out=v1, in_=view)
                    nc.vector.max_index(candp[:, c * CAP + 8:c * CAP + 16], v1, view)
                q = op_.tile([128, NSEL], U32, tag="q", name="q")
                for r in range(NSEL // 8):
                    v8 = cp.tile([128, 8], F32, tag="v8", name="v8")
                    nc.vector.max(out=v8, in_=cand)
                    nc.vector.max_index(q[:, r * 8:(r + 1) * 8], v8, cand)
                    nc.vector.match_replace(out=cand, in_to_replace=v8,
                                            in_values=cand, imm_value=SENT)
                nc.sync.dma_start(out=q56[t], in_=q)
                nc.sync.dma_start(out=p224[t], in_=candp)
    nc.compile()
    return nc


def _build_phase_b():
    nc = bacc.Bacc("TRN2", target_bir_lowering=False, debug=False,
                   enable_asserts=True, num_devices=NCORES)
    inp = nc.dram_tensor("inp", [NF, 128, FREE], F32, kind="ExternalInput")
    out = nc.dram_tensor("out", [5, 128, FREE], F32, kind="ExternalOutput")

    with tile.TileContext(nc) as tc:
        with tc.tile_pool(name="f", bufs=1) as fp, \
             tc.tile_pool(name="s", bufs=1) as sp:
            f = [fp.tile([128, FREE], F32, tag=f"f{i}", name=f"f{i}") for i in range(NF)]
            for i in range(NF):
                nc.sync.dma_start(out=f[i], in_=inp[i])
            (ax, ay, az, axh, axl, ayh, ayl, azh, azl, sqa,
             bx, by, bz, bxh, bxl, byh, byl, bzh, bzl, sqb) = f

            def T(tag):
                return sp.tile([128, FREE], F32, tag=tag, name=tag)

            def tt(o, a, b, op):
                nc.vector.tensor_tensor(out=o, in0=a, in1=b, op=op)

            def fma(a, b, ah, al, bh, bl, c, outt):
                # outt = round(a*b + c), bit-exact (Dekker product + TwoSum)
                ph = T("ph"); tt(ph, a, b, OP.mult)
                w = T("w"); e = T("e")
                tt(w, ah, bh, OP.mult)
                tt(e, w, ph, OP.subtract)
                tt(w, ah, bl, OP.mult)
                tt(e, e, w, OP.add)
                tt(w, al, bh, OP.mult)
                tt(e, e, w, OP.add)
                tt(w, al, bl, OP.mult)
                tt(e, e, w, OP.add)           # e = exact residual of a*b
                s = T("s"); tt(s, ph, c, OP.add)
                bv = T("bv"); tt(bv, s, ph, OP.subtract)
                ta_ = T("ta_"); tt(ta_, c, bv, OP.subtract)
                av = T("av"); tt(av, s, bv, OP.subtract)
                tb_ = T("tb_"); tt(tb_, ph, av, OP.subtract)
                tt(ta_, ta_, tb_, OP.add)     # exact residual of ph + c
                tt(ta_, ta_, e, OP.add)
                tt(outt, s, ta_, OP.add)

            m0 = T("m0"); tt(m0, ax, bx, OP.mult)
            f1 = T("f1"); fma(ay, by, ayh, ayl, byh, byl, m0, f1)
            dot = T("dot"); fma(az, bz, azh, azl, bzh, bzl, f1, dot)

            d2 = sp.tile([128, FREE], F32, tag="d2", name="d2")
            tt(d2, sqa, sqb, OP.add)
            nc.vector.tensor_scalar(dot, dot, 2.0, scalar2=None, op0=OP.mult)
            tt(d2, d2, dot, OP.subtract)
            nc.vector.tensor_scalar(d2, d2, 0.0, scalar2=None, op0=OP.max)

            vx = sp.tile([128, FREE], F32, tag="vx", name="vx"); tt(vx, bx, ax, OP.subtract)
            vy = sp.tile([128, FREE], F32, tag="vy", name="vy"); tt(vy, by, ay, OP.subtract)
            vz = sp.tile([128, FREE], F32, tag="vz", name="vz"); tt(vz, bz, az, OP.subtract)
            dd = sp.tile([128, FREE], F32, tag="dd", name="dd")
            t1 = T("sq1"); tt(t1, vx, vx, OP.mult)
            t2 = T("sq2"); tt(t2, vy, vy, OP.mult)
            tt(dd, t1, t2, OP.add)
            tt(t1, vz, vz, OP.mult)
            tt(dd, dd, t1, OP.add)

            for i, tl in enumerate([d2, dd, vx, vy, vz]):
                nc.sync.dma_start(out=out[i], in_=tl)
    nc.compile()
    return nc


# ---------------- host glue ----------------
def _host_sq(pos):
    x, y, z = pos[:, 0], pos[:, 1], pos[:, 2]
    return (x * x + y * y) + z * z


def _host_prep(pos):
    sq = _host_sq(pos)
    zord = np.argsort(pos[:, 2], kind="stable").astype(np.int64)
    posz = pos[zord]
    # Phase A works on box-centered coords: same exact-math d2, ~4x smaller
    # magnitudes -> ~4x smaller PE fp32 rounding noise -> larger rank margins.
    posc = posz - np.float32(28.0)
    sqc = _host_sq(posc)
    zs = posz[:, 2].astype(np.float64)
    lo = np.zeros(NTILES, dtype=np.int64)
    for t in range(NTILES):
        l = np.searchsorted(zs, zs[t * 128] - 5.0, side="left")
        h = np.searchsorted(zs, zs[t * 128 + 127] + 5.0, side="right")
        if h - l > W:
            l = max(0, (l + h - W) // 2)   # best effort (should not happen)
        lo[t] = min(l, N - W)
    ab = np.zeros((NTILES, 5, 128 + W), dtype=np.float32)
    for t in range(NTILES):
        q = posc[t * 128:(t + 1) * 128]
        ab[t, 0, :128] = np.float32(2.0) * q[:, 0]
        ab[t, 1, :128] = np.float32(2.0) * q[:, 1]
        ab[t, 2, :128] = np.float32(2.0) * q[:, 2]
        ab[t, 3, :128] = 1.0
        ab[t, 4, :128] = -sqc[t * 128:(t + 1) * 128]
        w = posc[lo[t]:lo[t] + W]
        ab[t, 0, 128:] = w[:, 0]
        ab[t, 1, 128:] = w[:, 1]
        ab[t, 2, 128:] = w[:, 2]
        ab[t, 3, 128:] = -sqc[lo[t]:lo[t] + W]
        ab[t, 4, 128:] = 1.0
    return dict(pos=pos, sq=sq, zord=zord, lo=lo, ab=ab)


def _decode_phase_a(prep, q56_all, p224_all):
    lo = prep["lo"]; zord = prep["zord"]
    cand_j = np.empty((N, NKEEP), dtype=np.int64)
    fallback = 0
    for t in range(NTILES):
        q = q56_all[t].astype(np.int64)
        p = p224_all[t].astype(np.int64)
        c = q // CAP
        pp = np.take_along_axis(p, q, axis=1)
        gsorted = lo[t] + c + pp * S
        selfidx = (t * 128 + np.arange(128))[:, None]
        notself = gsorted != selfidx
        for r in range(128):
            g = gsorted[r][notself[r]]
            _, first = np.unique(g, return_index=True)
            g = g[np.sort(first)]
            if len(g) < NKEEP:      # should not happen; pad defensively
                fallback += 1
                padv = [v for v in range(lo[t], lo[t] + W)
                        if v not in set(g) and v != t * 128 + r][:NKEEP - len(g)]
                g = np.concatenate([g, np.array(padv, dtype=np.int64)])
            cand_j[t * 128 + r] = zord[g[:NKEEP]]
    return cand_j


def _split(v):
    c = np.float32(4097.0)
    t = np.float32(c * v)
    hi = np.float32(t - np.float32(t - v))
    return hi, np.float32(v - hi)


def _phase_b_inputs(prep, cand_j, core):
    pos = prep["pos"]; sq = prep["sq"]; zord = prep["zord"]
    rows_orig = zord[np.arange(core * RPC, (core + 1) * RPC)]
    cj = cand_j[core * RPC:(core + 1) * RPC]
    a = pos[rows_orig]
    b = pos[cj]
    arr = np.empty((NF, 128, FREE), dtype=np.float32)

    def fill(fi, vals):
        arr[fi] = vals.reshape(TPC, 128, NKEEP).transpose(1, 0, 2).reshape(128, FREE)

    for d in range(3):
        h, l = _split(a[:, d])
        fill(d, np.broadcast_to(a[:, d][:, None], (RPC, NKEEP)))
        fill(3 + 2 * d, np.broadcast_to(h[:, None], (RPC, NKEEP)))
        fill(4 + 2 * d, np.broadcast_to(l[:, None], (RPC, NKEEP)))
        h, l = _split(b[:, :, d])
        fill(10 + d, b[:, :, d])
        fill(13 + 2 * d, h)
        fill(14 + 2 * d, l)
    fill(9, np.broadcast_to(sq[rows_orig][:, None], (RPC, NKEEP)))
    fill(19, sq[cj])
    return arr


def _decode_phase_b(outs):
    full = []
    for fi in range(5):
        per = [outs[c][fi].reshape(128, TPC, NKEEP).transpose(1, 0, 2).reshape(RPC, NKEEP)
               for c in range(NCORES)]
        full.append(np.concatenate(per, axis=0))
    return full


# ---------------- entry point ----------------
def kernel(pos):
    pos = np.ascontiguousarray(np.asarray(pos, dtype=np.float32))
    assert pos.shape == (N, 3)
    prep = _host_prep(pos)

    if "a" not in _CACHE:
        _CACHE["a"] = _build_phase_a()
    nca = _CACHE["a"]
    in_a = [{"ab": np.ascontiguousarray(prep["ab"][c * TPC:(c + 1) * TPC])}
            for c in range(NCORES)]
    res_a = run_bass_kernel_spmd(nca, in_a, list(range(NCORES)))
    q56 = np.stack([res_a.results[c]["q56"] for c in range(NCORES)]).reshape(NTILES, 128, NSEL)
    p224 = np.stack([res_a.results[c]["p224"] for c in range(NCORES)]).reshape(NTILES, 128, NCAND)

    cand_j = _decode_phase_a(prep, q56, p224)

    if "b" not in _CACHE:
        _CACHE["b"] = _build_phase_b()
    ncb = _CACHE["b"]
    in_b = [{"inp": _phase_b_inputs(prep, cand_j, c)} for c in range(NCORES)]
    res_b = run_bass_kernel_spmd(ncb, in_b, list(range(NCORES)))
    d2c, ddc, vxc, vyc, vzc = _decode_phase_b([res_b.results[c]["out"] for c in range(NCORES)])

    # final selection: sort 48 by (d2 asc, j asc), take K, mask by cutoff
    order = np.lexsort((cand_j, d2c), axis=-1)[:, :K]
    d2s = np.take_along_axis(d2c, order, axis=1)
    js = np.take_along_axis(cand_j, order, axis=1)
    dds = np.take_along_axis(ddc, order, axis=1)
    vxs = np.take_along_axis(vxc, order, axis=1)
    vys = np.take_along_axis(vyc, order, axis=1)
    vzs = np.take_along_axis(vzc, order, axis=1)
    valid = d2s <= CUTOFF2
    rows_orig = prep["zord"]
    ii = rows_orig[:, None]
    src_s = np.where(valid, js, ii)
    w_s = np.where(valid, np.sqrt(dds, dtype=np.float32), np.float32(0.0))
    vx_s = np.where(valid, vxs, np.float32(0.0))
    vy_s = np.where(valid, vys, np.float32(0.0))
    vz_s = np.where(valid, vzs, np.float32(0.0))

    inv = np.empty(N, dtype=np.int64); inv[rows_orig] = np.arange(N)
    src = src_s[inv].reshape(-1).astype(np.int32)
    dst = np.repeat(np.arange(N, dtype=np.int32), K)
    w = w_s[inv].reshape(-1)
    vec = np.stack([vx_s[inv].reshape(-1), vy_s[inv].reshape(-1), vz_s[inv].reshape(-1)], axis=1)

    ar = np.arange(N, dtype=np.int32)
    edge_index = np.stack([np.concatenate([src, ar]), np.concatenate([dst, ar])])
    edge_weight = np.concatenate([w, np.zeros(N, np.float32)])
    edge_vec = np.concatenate([vec, np.zeros((N, 3), np.float32)], axis=0)
    return edge_index, edge_weight, edge_vec


# revision 5
# speedup vs baseline: 1.1258x; 1.1258x over previous
"""Deterministic radius-graph KNN (N=16384, K=32, cutoff 5.0) on 8 trn2 NeuronCores.

Strategy (data-parallel over query atoms, z-slab sharded):
  Host: z-sort atoms; per 128-query tile build a 3584-wide z-window of candidates.
  Phase A (device): -d2 approx via PE fp32 matmul (K=5 fused: 2*dot - sq_i - sq_j)
    on box-centered coords; the window column id is embedded in the low 12
    mantissa bits (DVE bitwise and/or with iota constants), then hierarchical
    top-56 selection per query (14 strided chunks x cap-16 via max8/match_replace
    only -- no max_index needed, indices decode from the value bits).
  Host: decode candidate indices from key bits, gather candidate fields.
  Phase B (device): bit-exact XLA:CPU d2/dd/vec for the 48 best candidates per
    query, via Dekker-split + TwoSum fp32 emulation of the CPU's fma-chain dot.
  Host: final sort of 48 by (d2, j), cutoff mask, self-pair padding, assembly.

The reference's top-k ordering is knife-edge sensitive to d2 rounding, so phase B
reproduces XLA:CPU arithmetic bit-for-bit; the final output matches the oracle
bitwise on uniform inputs.
"""
import numpy as np

import concourse.bacc as bacc
import concourse.mybir as mybir
import concourse.tile as tile
from concourse.bass_utils import run_bass_kernel_spmd

# ---------------- constants (hardcoded for the given problem) ----------------
N = 16384
K = 32
CUTOFF2 = np.float32(25.0)
NCORES = 8
RPC = N // NCORES          # queries per core
TPC = RPC // 128           # row-tiles per core
NTILES = N // 128
W = 3584                   # candidate window (= 7*512 = 14*256)
S = 14                     # strided chunks per window
CAP = 16                   # survivors per chunk
NCAND = S * CAP            # 224
NSEL = 56                  # stage-2 extracted (incl. self)
NKEEP = 48                 # candidates refined in phase B
SENT = -3.0e38
FREE = TPC * NKEEP
NF = 20

F32 = mybir.dt.float32
U32 = mybir.dt.uint32
I32 = mybir.dt.int32
OP = mybir.AluOpType

_CACHE = {}


# ---------------- device programs ----------------
def _build_phase_a():
    nc = bacc.Bacc("TRN2", target_bir_lowering=False, debug=False,
                   enable_asserts=True, num_devices=NCORES)
    ab = nc.dram_tensor("ab", [TPC, 5, 128 + W], F32, kind="ExternalInput")
    k56 = nc.dram_tensor("k56", [TPC, 128, NSEL], F32, kind="ExternalOutput")

    with tile.TileContext(nc) as tc:
        with tc.tile_pool(name="const", bufs=1) as kp, \
             tc.tile_pool(name="inp", bufs=3) as inp, \
             tc.tile_pool(name="big", bufs=2) as big, \
             tc.tile_pool(name="cand", bufs=2) as cp, \
             tc.tile_pool(name="out", bufs=2) as op_, \
             tc.tile_pool(name="psum", bufs=4, space="PSUM") as pp:
            # keying constants: low-12-bit mask + window column id
            kmask = kp.tile([128, W], I32, name="kmask")
            kcol = kp.tile([128, W], I32, name="kcol")
            nc.gpsimd.iota(kmask, pattern=[[0, W]], base=-4096, channel_multiplier=0)
            nc.gpsimd.iota(kcol, pattern=[[1, W]], base=0, channel_multiplier=0)
            for t in range(TPC):
                tab = inp.tile([5, 128 + W], F32, tag="tab", name="tab")
                nc.sync.dma_start(out=tab, in_=ab[t])
                neg = big.tile([128, W], F32, tag="neg", name="neg")
                for b in range(W // 512):
                    ps = pp.tile([128, 512], F32, tag="ps", name="ps")
                    nc.tensor.matmul(ps, tab[:, :128],
                                     tab[:, 128 + b * 512:128 + (b + 1) * 512],
                                     start=True, stop=True)
                    nc.scalar.activation(neg[:, b * 512:(b + 1) * 512], ps,
                                         mybir.ActivationFunctionType.Copy)
                # embed window column into low 12 mantissa bits (values are
                # approximate anyway; shifts contenders by <= ~4e-3)
                negU = neg.bitcast(I32)
                nc.vector.tensor_tensor(out=negU, in0=negU, in1=kmask, op=OP.bitwise_and)
                nc.vector.tensor_tensor(out=negU, in0=negU, in1=kcol, op=OP.bitwise_or)
                cand = cp.tile([128, NCAND], F32, tag="cand", name="cand")
                for c in range(S):
                    view = neg[:, c:W:S]
                    v0 = cand[:, c * CAP:c * CAP + 8]
                    nc.vector.max(out=v0, in_=view)
                    nc.vector.match_replace(out=view, in_to_replace=v0,
                                            in_values=view, imm_value=SENT)
                    nc.vector.max(out=cand[:, c * CAP + 8:c * CAP + 16], in_=view)
                q = op_.tile([128, NSEL], F32, tag="q", name="q")
                for r in range(NSEL // 8):
                    nc.vector.max(out=q[:, r * 8:(r + 1) * 8], in_=cand)
                    if r < NSEL // 8 - 1:
                        nc.vector.match_replace(out=cand,
                                                in_to_replace=q[:, r * 8:(r + 1) * 8],
                                                in_values=cand, imm_value=SENT)
                nc.sync.dma_start(out=k56[t], in_=q)
    nc.compile()
    return nc


def _build_phase_b():
    nc = bacc.Bacc("TRN2", target_bir_lowering=False, debug=False,
                   enable_asserts=True, num_devices=NCORES)
    inp = nc.dram_tensor("inp", [NF, 128, FREE], F32, kind="ExternalInput")
    out = nc.dram_tensor("out", [5, 128, FREE], F32, kind="ExternalOutput")

    with tile.TileContext(nc) as tc:
        with tc.tile_pool(name="f", bufs=1) as fp, \
             tc.tile_pool(name="s", bufs=1) as sp:
            f = [fp.tile([128, FREE], F32, tag=f"f{i}", name=f"f{i}") for i in range(NF)]
            for i in range(NF):
                nc.sync.dma_start(out=f[i], in_=inp[i])
            (ax, ay, az, axh, axl, ayh, ayl, azh, azl, sqa,
             bx, by, bz, bxh, bxl, byh, byl, bzh, bzl, sqb) = f

            def T(tag):
                return sp.tile([128, FREE], F32, tag=tag, name=tag)

            def tt(o, a, b, op):
                nc.vector.tensor_tensor(out=o, in0=a, in1=b, op=op)

            def fma(a, b, ah, al, bh, bl, c, outt):
                # outt = round(a*b + c), bit-exact (Dekker product + TwoSum)
                ph = T("ph"); tt(ph, a, b, OP.mult)
                w = T("w"); e = T("e")
                tt(w, ah, bh, OP.mult)
                tt(e, w, ph, OP.subtract)
                tt(w, ah, bl, OP.mult)
                tt(e, e, w, OP.add)
                tt(w, al, bh, OP.mult)
                tt(e, e, w, OP.add)
                tt(w, al, bl, OP.mult)
                tt(e, e, w, OP.add)           # e = exact residual of a*b
                s = T("s"); tt(s, ph, c, OP.add)
                bv = T("bv"); tt(bv, s, ph, OP.subtract)
                ta_ = T("ta_"); tt(ta_, c, bv, OP.subtract)
                av = T("av"); tt(av, s, bv, OP.subtract)
                tb_ = T("tb_"); tt(tb_, ph, av, OP.subtract)
                tt(ta_, ta_, tb_, OP.add)     # exact residual of ph + c
                tt(ta_, ta_, e, OP.add)
                tt(outt, s, ta_, OP.add)

            m0 = T("m0"); tt(m0, ax, bx, OP.mult)
            f1 = T("f1"); fma(ay, by, ayh, ayl, byh, byl, m0, f1)
            dot = T("dot"); fma(az, bz, azh, azl, bzh, bzl, f1, dot)

            d2 = sp.tile([128, FREE], F32, tag="d2", name="d2")
            tt(d2, sqa, sqb, OP.add)
            nc.vector.tensor_scalar(dot, dot, 2.0, scalar2=None, op0=OP.mult)
            tt(d2, d2, dot, OP.subtract)
            nc.vector.tensor_scalar(d2, d2, 0.0, scalar2=None, op0=OP.max)

            vx = sp.tile([128, FREE], F32, tag="vx", name="vx"); tt(vx, bx, ax, OP.subtract)
            vy = sp.tile([128, FREE], F32, tag="vy", name="vy"); tt(vy, by, ay, OP.subtract)
            vz = sp.tile([128, FREE], F32, tag="vz", name="vz"); tt(vz, bz, az, OP.subtract)
            dd = sp.tile([128, FREE], F32, tag="dd", name="dd")
            t1 = T("sq1"); tt(t1, vx, vx, OP.mult)
            t2 = T("sq2"); tt(t2, vy, vy, OP.mult)
            tt(dd, t1, t2, OP.add)
            tt(t1, vz, vz, OP.mult)
            tt(dd, dd, t1, OP.add)

            for i, tl in enumerate([d2, dd, vx, vy, vz]):
                nc.sync.dma_start(out=out[i], in_=tl)
    nc.compile()
    return nc


# ---------------- host glue ----------------
def _host_sq(pos):
    x, y, z = pos[:, 0], pos[:, 1], pos[:, 2]
    return (x * x + y * y) + z * z


def _host_prep(pos):
    sq = _host_sq(pos)
    zord = np.argsort(pos[:, 2], kind="stable").astype(np.int64)
    posz = pos[zord]
    # Phase A works on box-centered coords: same exact-math d2, ~4x smaller
    # magnitudes -> ~4x smaller PE fp32 rounding noise -> larger rank margins.
    posc = posz - np.float32(28.0)
    sqc = _host_sq(posc)
    zs = posz[:, 2].astype(np.float64)
    lo = np.zeros(NTILES, dtype=np.int64)
    for t in range(NTILES):
        l = np.searchsorted(zs, zs[t * 128] - 5.0, side="left")
        h = np.searchsorted(zs, zs[t * 128 + 127] + 5.0, side="right")
        if h - l > W:
            l = max(0, (l + h - W) // 2)   # best effort (should not happen)
        lo[t] = min(l, N - W)
    ab = np.zeros((NTILES, 5, 128 + W), dtype=np.float32)
    for t in range(NTILES):
        q = posc[t * 128:(t + 1) * 128]
        ab[t, 0, :128] = np.float32(2.0) * q[:, 0]
        ab[t, 1, :128] = np.float32(2.0) * q[:, 1]
        ab[t, 2, :128] = np.float32(2.0) * q[:, 2]
        ab[t, 3, :128] = 1.0
        ab[t, 4, :128] = -sqc[t * 128:(t + 1) * 128]
        w = posc[lo[t]:lo[t] + W]
        ab[t, 0, 128:] = w[:, 0]
        ab[t, 1, 128:] = w[:, 1]
        ab[t, 2, 128:] = w[:, 2]
        ab[t, 3, 128:] = -sqc[lo[t]:lo[t] + W]
        ab[t, 4, 128:] = 1.0
    return dict(pos=pos, sq=sq, zord=zord, lo=lo, ab=ab)


def _decode_phase_a(prep, k56_all):
    """k56_all: [NTILES,128,NSEL] keyed f32 -> cand_j [N, NKEEP] original indices."""
    lo = prep["lo"]; zord = prep["zord"]
    bits = k56_all.view(np.uint32)
    isneg = (bits >> 31) == 1                     # real candidates (self is +0-keyed)
    wincol = (bits & np.uint32(0xFFF)).astype(np.int64)
    cand_j = np.empty((N, NKEEP), dtype=np.int64)
    for t in range(NTILES):
        gs = lo[t] + wincol[t]
        selfidx = (t * 128 + np.arange(128))[:, None]
        ok = isneg[t] & (gs != selfidx)
        for r in range(128):
            g = gs[r][ok[r]]
            _, first = np.unique(g, return_index=True)
            g = g[np.sort(first)]
            if len(g) < NKEEP:       # should not happen; pad defensively
                pool = [v for v in range(lo[t], lo[t] + W)
                        if v not in set(g) and v != t * 128 + r][:NKEEP - len(g)]
                g = np.concatenate([g, np.array(pool, dtype=np.int64)])
            cand_j[t * 128 + r] = zord[g[:NKEEP]]
    return cand_j


def _split(v):
    c = np.float32(4097.0)
    t = np.float32(c * v)
    hi = np.float32(t - np.float32(t - v))
    return hi, np.float32(v - hi)


def _phase_b_inputs(prep, cand_j, core):
    pos = prep["pos"]; sq = prep["sq"]; zord = prep["zord"]
    rows_orig = zord[np.arange(core * RPC, (core + 1) * RPC)]
    cj = cand_j[core * RPC:(core + 1) * RPC]
    a = pos[rows_orig]
    b = pos[cj]
    arr = np.empty((NF, 128, FREE), dtype=np.float32)

    def fill(fi, vals):
        arr[fi] = vals.reshape(TPC, 128, NKEEP).transpose(1, 0, 2).reshape(128, FREE)

    for d in range(3):
        h, l = _split(a[:, d])
        fill(d, np.broadcast_to(a[:, d][:, None], (RPC, NKEEP)))
        fill(3 + 2 * d, np.broadcast_to(h[:, None], (RPC, NKEEP)))
        fill(4 + 2 * d, np.broadcast_to(l[:, None], (RPC, NKEEP)))
        h, l = _split(b[:, :, d])
        fill(10 + d, b[:, :, d])
        fill(13 + 2 * d, h)
        fill(14 + 2 * d, l)
    fill(9, np.broadcast_to(sq[rows_orig][:, None], (RPC, NKEEP)))
    fill(19, sq[cj])
    return arr


def _decode_phase_b(outs):
    full = []
    for fi in range(5):
        per = [outs[c][fi].reshape(128, TPC, NKEEP).transpose(1, 0, 2).reshape(RPC, NKEEP)
               for c in range(NCORES)]
        full.append(np.concatenate(per, axis=0))
    return full


# ---------------- entry point ----------------
def kernel(pos):
    pos = np.ascontiguousarray(np.asarray(pos, dtype=np.float32))
    assert pos.shape == (N, 3)
    prep = _host_prep(pos)

    if "a" not in _CACHE:
        _CACHE["a"] = _build_phase_a()
    nca = _CACHE["a"]
    in_a = [{"ab": np.ascontiguousarray(prep["ab"][c * TPC:(c + 1) * TPC])}
            for c in range(NCORES)]
    res_a = run_bass_kernel_spmd(nca, in_a, list(range(NCORES)))
    k56 = np.stack([res_a.results[c]["k56"] for c in range(NCORES)]).reshape(NTILES, 128, NSEL)

    cand_j = _decode_phase_a(prep, k56)

    if "b" not in _CACHE:
        _CACHE["b"] = _build_phase_b()
    ncb = _CACHE["b"]
    in_b = [{"inp": _phase_b_inputs(prep, cand_j, c)} for c in range(NCORES)]
    res_b = run_bass_kernel_spmd(ncb, in_b, list(range(NCORES)))
    d2c, ddc, vxc, vyc, vzc = _decode_phase_b([res_b.results[c]["out"] for c in range(NCORES)])

    # final selection: sort 48 by (d2 asc, j asc), take K, mask by cutoff
    order = np.lexsort((cand_j, d2c), axis=-1)[:, :K]
    d2s = np.take_along_axis(d2c, order, axis=1)
    js = np.take_along_axis(cand_j, order, axis=1)
    dds = np.take_along_axis(ddc, order, axis=1)
    vxs = np.take_along_axis(vxc, order, axis=1)
    vys = np.take_along_axis(vyc, order, axis=1)
    vzs = np.take_along_axis(vzc, order, axis=1)
    valid = d2s <= CUTOFF2
    rows_orig = prep["zord"]
    ii = rows_orig[:, None]
    src_s = np.where(valid, js, ii)
    w_s = np.where(valid, np.sqrt(dds, dtype=np.float32), np.float32(0.0))
    vx_s = np.where(valid, vxs, np.float32(0.0))
    vy_s = np.where(valid, vys, np.float32(0.0))
    vz_s = np.where(valid, vzs, np.float32(0.0))

    inv = np.empty(N, dtype=np.int64); inv[rows_orig] = np.arange(N)
    src = src_s[inv].reshape(-1).astype(np.int32)
    dst = np.repeat(np.arange(N, dtype=np.int32), K)
    w = w_s[inv].reshape(-1)
    vec = np.stack([vx_s[inv].reshape(-1), vy_s[inv].reshape(-1), vz_s[inv].reshape(-1)], axis=1)

    ar = np.arange(N, dtype=np.int32)
    edge_index = np.stack([np.concatenate([src, ar]), np.concatenate([dst, ar])])
    edge_weight = np.concatenate([w, np.zeros(N, np.float32)])
    edge_vec = np.concatenate([vec, np.zeros((N, 3), np.float32)], axis=0)
    return edge_index, edge_weight, edge_vec


# revision 6
# speedup vs baseline: 1.3346x; 1.1854x over previous
"""Deterministic radius-graph KNN (N=16384, K=32, cutoff 5.0) on 8 trn2 NeuronCores.

Strategy (data-parallel over query atoms, z-slab sharded):
  Host: z-sort atoms; per 128-query tile build a 3584-wide z-window of candidates.
  Phase A (device): -d2 approx via PE fp32 matmul (K=5 fused: 2*dot - sq_i - sq_j)
    on box-centered coords; the window column id is embedded in the low 12
    mantissa bits (DVE bitwise and/or with iota constants), then hierarchical
    top-56 selection per query (14 strided chunks x cap-16 via max8/match_replace
    only -- no max_index needed, indices decode from the value bits).
  Host: decode candidate indices from key bits, gather candidate fields.
  Phase B (device): bit-exact XLA:CPU d2/dd/vec for the 48 best candidates per
    query, via Dekker-split + TwoSum fp32 emulation of the CPU's fma-chain dot.
  Host: final sort of 48 by (d2, j), cutoff mask, self-pair padding, assembly.

The reference's top-k ordering is knife-edge sensitive to d2 rounding, so phase B
reproduces XLA:CPU arithmetic bit-for-bit; the final output matches the oracle
bitwise on uniform inputs.
"""
import numpy as np

import concourse.bacc as bacc
import concourse.mybir as mybir
import concourse.tile as tile
from concourse.bass_utils import run_bass_kernel_spmd

# ---------------- constants (hardcoded for the given problem) ----------------
N = 16384
K = 32
CUTOFF2 = np.float32(25.0)
NCORES = 8
RPC = N // NCORES          # queries per core
TPC = RPC // 128           # row-tiles per core
NTILES = N // 128
W = 3584                   # candidate window (= 7*512 = 14*256)
S = 14                     # strided chunks per window
CAP = 16                   # survivors per chunk
NCAND = S * CAP            # 224
NSEL = 48                  # stage-2 extracted (incl. self)
NKEEP = 47                 # candidates refined in phase B
SENT = -3.0e38
FREE = TPC * NKEEP
NF = 20

F32 = mybir.dt.float32
U32 = mybir.dt.uint32
I32 = mybir.dt.int32
F16 = mybir.dt.float16
OP = mybir.AluOpType

_CACHE = {}


# ---------------- device programs ----------------
def _build_phase_a():
    nc = bacc.Bacc("TRN2", target_bir_lowering=False, debug=False,
                   enable_asserts=True, num_devices=NCORES)
    ab = nc.dram_tensor("ab", [TPC, 5, 128 + W], F32, kind="ExternalInput")
    k48 = nc.dram_tensor("k48", [TPC, 128, NSEL], F32, kind="ExternalOutput")

    with tile.TileContext(nc) as tc:
        with tc.tile_pool(name="const", bufs=1) as kp, \
             tc.tile_pool(name="inp", bufs=3) as inp, \
             tc.tile_pool(name="big", bufs=2) as big, \
             tc.tile_pool(name="cand", bufs=2) as cp, \
             tc.tile_pool(name="out", bufs=2) as op_, \
             tc.tile_pool(name="psum", bufs=4, space="PSUM") as pp:
            # keying constant: window column id (low 12 mantissa bits)
            kcol = kp.tile([128, W], I32, name="kcol")
            nc.gpsimd.iota(kcol, pattern=[[1, W]], base=0, channel_multiplier=0)
            for t in range(TPC):
                tab = inp.tile([5, 128 + W], F32, tag="tab", name="tab")
                nc.sync.dma_start(out=tab, in_=ab[t])
                h16 = big.tile([128, W], F16, tag="h16", name="h16")
                neg = big.tile([128, W], F32, tag="neg", name="neg")
                for b in range(W // 512):
                    ps = pp.tile([128, 512], F32, tag="ps", name="ps")
                    nc.tensor.matmul(ps, tab[:, :128],
                                     tab[:, 128 + b * 512:128 + (b + 1) * 512],
                                     start=True, stop=True)
                    # fp16 round-trip on ACT zeroes the low 13 mantissa bits
                    # (quantizes the approx values by <=2^-11 rel; fine), so the
                    # DVE only needs one bitwise-or to embed the column id.
                    nc.scalar.activation(h16[:, b * 512:(b + 1) * 512], ps,
                                         mybir.ActivationFunctionType.Copy)
                nc.scalar.activation(neg, h16, mybir.ActivationFunctionType.Copy)
                negU = neg.bitcast(I32)
                nc.vector.tensor_tensor(out=negU, in0=negU, in1=kcol, op=OP.bitwise_or)
                cand = cp.tile([128, NCAND], F32, tag="cand", name="cand")
                for c in range(S):
                    view = neg[:, c:W:S]
                    v0 = cand[:, c * CAP:c * CAP + 8]
                    nc.vector.max(out=v0, in_=view)
                    nc.vector.match_replace(out=view, in_to_replace=v0,
                                            in_values=view, imm_value=SENT)
                    nc.vector.max(out=cand[:, c * CAP + 8:c * CAP + 16], in_=view)
                q = op_.tile([128, NSEL], F32, tag="q", name="q")
                for r in range(NSEL // 8):
                    nc.vector.max(out=q[:, r * 8:(r + 1) * 8], in_=cand)
                    if r < NSEL // 8 - 1:
                        nc.vector.match_replace(out=cand,
                                                in_to_replace=q[:, r * 8:(r + 1) * 8],
                                                in_values=cand, imm_value=SENT)
                nc.sync.dma_start(out=k48[t], in_=q)
    nc.compile()
    return nc


def _build_phase_b():
    nc = bacc.Bacc("TRN2", target_bir_lowering=False, debug=False,
                   enable_asserts=True, num_devices=NCORES)
    inp = nc.dram_tensor("inp", [NF, 128, FREE], F32, kind="ExternalInput")
    out = nc.dram_tensor("out", [5, 128, FREE], F32, kind="ExternalOutput")

    with tile.TileContext(nc) as tc:
        with tc.tile_pool(name="f", bufs=1) as fp, \
             tc.tile_pool(name="s", bufs=1) as sp:
            f = [fp.tile([128, FREE], F32, tag=f"f{i}", name=f"f{i}") for i in range(NF)]
            for i in range(NF):
                nc.sync.dma_start(out=f[i], in_=inp[i])
            (ax, ay, az, axh, axl, ayh, ayl, azh, azl, sqa,
             bx, by, bz, bxh, bxl, byh, byl, bzh, bzl, sqb) = f

            def T(tag):
                return sp.tile([128, FREE], F32, tag=tag, name=tag)

            def tt(o, a, b, op):
                nc.vector.tensor_tensor(out=o, in0=a, in1=b, op=op)

            def fma(a, b, ah, al, bh, bl, c, outt):
                # outt = round(a*b + c), bit-exact (Dekker product + TwoSum)
                ph = T("ph"); tt(ph, a, b, OP.mult)
                w = T("w"); e = T("e")
                tt(w, ah, bh, OP.mult)
                tt(e, w, ph, OP.subtract)
                tt(w, ah, bl, OP.mult)
                tt(e, e, w, OP.add)
                tt(w, al, bh, OP.mult)
                tt(e, e, w, OP.add)
                tt(w, al, bl, OP.mult)
                tt(e, e, w, OP.add)           # e = exact residual of a*b
                s = T("s"); tt(s, ph, c, OP.add)
                bv = T("bv"); tt(bv, s, ph, OP.subtract)
                ta_ = T("ta_"); tt(ta_, c, bv, OP.subtract)
                av = T("av"); tt(av, s, bv, OP.subtract)
                tb_ = T("tb_"); tt(tb_, ph, av, OP.subtract)
                tt(ta_, ta_, tb_, OP.add)     # exact residual of ph + c
                tt(ta_, ta_, e, OP.add)
                tt(outt, s, ta_, OP.add)

            m0 = T("m0"); tt(m0, ax, bx, OP.mult)
            f1 = T("f1"); fma(ay, by, ayh, ayl, byh, byl, m0, f1)
            dot = T("dot"); fma(az, bz, azh, azl, bzh, bzl, f1, dot)

            d2 = sp.tile([128, FREE], F32, tag="d2", name="d2")
            tt(d2, sqa, sqb, OP.add)
            nc.vector.tensor_scalar(dot, dot, 2.0, scalar2=None, op0=OP.mult)
            tt(d2, d2, dot, OP.subtract)
            nc.vector.tensor_scalar(d2, d2, 0.0, scalar2=None, op0=OP.max)

            def gt(o, a, b, op):
                nc.gpsimd.tensor_tensor(out=o, in0=a, in1=b, op=op)

            vx = sp.tile([128, FREE], F32, tag="vx", name="vx"); gt(vx, bx, ax, OP.subtract)
            vy = sp.tile([128, FREE], F32, tag="vy", name="vy"); gt(vy, by, ay, OP.subtract)
            vz = sp.tile([128, FREE], F32, tag="vz", name="vz"); gt(vz, bz, az, OP.subtract)
            dd = sp.tile([128, FREE], F32, tag="dd", name="dd")
            t1 = sp.tile([128, FREE], F32, tag="sq1", name="sq1"); gt(t1, vx, vx, OP.mult)
            t2 = sp.tile([128, FREE], F32, tag="sq2", name="sq2"); gt(t2, vy, vy, OP.mult)
            gt(dd, t1, t2, OP.add)
            gt(t1, vz, vz, OP.mult)
            gt(dd, dd, t1, OP.add)

            for i, tl in enumerate([d2, dd, vx, vy, vz]):
                nc.sync.dma_start(out=out[i], in_=tl)
    nc.compile()
    return nc


# ---------------- host glue ----------------
def _host_sq(pos):
    x, y, z = pos[:, 0], pos[:, 1], pos[:, 2]
    return (x * x + y * y) + z * z


def _host_prep(pos):
    sq = _host_sq(pos)
    zord = np.argsort(pos[:, 2], kind="stable").astype(np.int64)
    posz = pos[zord]
    # Phase A works on box-centered coords: same exact-math d2, ~4x smaller
    # magnitudes -> ~4x smaller PE fp32 rounding noise -> larger rank margins.
    posc = posz - np.float32(28.0)
    sqc = _host_sq(posc)
    zs = posz[:, 2].astype(np.float64)
    lo = np.zeros(NTILES, dtype=np.int64)
    for t in range(NTILES):
        l = np.searchsorted(zs, zs[t * 128] - 5.0, side="left")
        h = np.searchsorted(zs, zs[t * 128 + 127] + 5.0, side="right")
        if h - l > W:
            l = max(0, (l + h - W) // 2)   # best effort (should not happen)
        lo[t] = min(l, N - W)
    ab = np.zeros((NTILES, 5, 128 + W), dtype=np.float32)
    for t in range(NTILES):
        q = posc[t * 128:(t + 1) * 128]
        ab[t, 0, :128] = np.float32(2.0) * q[:, 0]
        ab[t, 1, :128] = np.float32(2.0) * q[:, 1]
        ab[t, 2, :128] = np.float32(2.0) * q[:, 2]
        ab[t, 3, :128] = 1.0
        ab[t, 4, :128] = -sqc[t * 128:(t + 1) * 128]
        w = posc[lo[t]:lo[t] + W]
        ab[t, 0, 128:] = w[:, 0]
        ab[t, 1, 128:] = w[:, 1]
        ab[t, 2, 128:] = w[:, 2]
        ab[t, 3, 128:] = -sqc[lo[t]:lo[t] + W]
        ab[t, 4, 128:] = 1.0
    return dict(pos=pos, sq=sq, zord=zord, lo=lo, ab=ab)


def _decode_phase_a(prep, k48_all):
    """k48_all: [NTILES,128,NSEL] keyed f32 -> cand_j [N, NKEEP] original indices."""
    lo = prep["lo"]; zord = prep["zord"]
    bits = k48_all.view(np.uint32)
    isneg = (bits >> 31) == 1                     # real candidates (self is +0-keyed)
    wincol = (bits & np.uint32(0xFFF)).astype(np.int64)
    cand_j = np.empty((N, NKEEP), dtype=np.int64)
    for t in range(NTILES):
        gs = lo[t] + wincol[t]
        selfidx = (t * 128 + np.arange(128))[:, None]
        ok = isneg[t] & (gs != selfidx)
        for r in range(128):
            g = gs[r][ok[r]]
            _, first = np.unique(g, return_index=True)
            g = g[np.sort(first)]
            if len(g) < NKEEP:       # should not happen; pad defensively
                pool = [v for v in range(lo[t], lo[t] + W)
                        if v not in set(g) and v != t * 128 + r][:NKEEP - len(g)]
                g = np.concatenate([g, np.array(pool, dtype=np.int64)])
            cand_j[t * 128 + r] = zord[g[:NKEEP]]
    return cand_j


def _split(v):
    c = np.float32(4097.0)
    t = np.float32(c * v)
    hi = np.float32(t - np.float32(t - v))
    return hi, np.float32(v - hi)


def _phase_b_inputs(prep, cand_j, core):
    pos = prep["pos"]; sq = prep["sq"]; zord = prep["zord"]
    rows_orig = zord[np.arange(core * RPC, (core + 1) * RPC)]
    cj = cand_j[core * RPC:(core + 1) * RPC]
    a = pos[rows_orig]
    b = pos[cj]
    arr = np.empty((NF, 128, FREE), dtype=np.float32)

    def fill(fi, vals):
        arr[fi] = vals.reshape(TPC, 128, NKEEP).transpose(1, 0, 2).reshape(128, FREE)

    for d in range(3):
        h, l = _split(a[:, d])
        fill(d, np.broadcast_to(a[:, d][:, None], (RPC, NKEEP)))
        fill(3 + 2 * d, np.broadcast_to(h[:, None], (RPC, NKEEP)))
        fill(4 + 2 * d, np.broadcast_to(l[:, None], (RPC, NKEEP)))
        h, l = _split(b[:, :, d])
        fill(10 + d, b[:, :, d])
        fill(13 + 2 * d, h)
        fill(14 + 2 * d, l)
    fill(9, np.broadcast_to(sq[rows_orig][:, None], (RPC, NKEEP)))
    fill(19, sq[cj])
    return arr


def _decode_phase_b(outs):
    full = []
    for fi in range(5):
        per = [outs[c][fi].reshape(128, TPC, NKEEP).transpose(1, 0, 2).reshape(RPC, NKEEP)
               for c in range(NCORES)]
        full.append(np.concatenate(per, axis=0))
    return full


# ---------------- entry point ----------------
def kernel(pos):
    pos = np.ascontiguousarray(np.asarray(pos, dtype=np.float32))
    assert pos.shape == (N, 3)
    prep = _host_prep(pos)

    if "a" not in _CACHE:
        _CACHE["a"] = _build_phase_a()
    nca = _CACHE["a"]
    in_a = [{"ab": np.ascontiguousarray(prep["ab"][c * TPC:(c + 1) * TPC])}
            for c in range(NCORES)]
    res_a = run_bass_kernel_spmd(nca, in_a, list(range(NCORES)))
    k48 = np.stack([res_a.results[c]["k48"] for c in range(NCORES)]).reshape(NTILES, 128, NSEL)

    cand_j = _decode_phase_a(prep, k48)

    if "b" not in _CACHE:
        _CACHE["b"] = _build_phase_b()
    ncb = _CACHE["b"]
    in_b = [{"inp": _phase_b_inputs(prep, cand_j, c)} for c in range(NCORES)]
    res_b = run_bass_kernel_spmd(ncb, in_b, list(range(NCORES)))
    d2c, ddc, vxc, vyc, vzc = _decode_phase_b([res_b.results[c]["out"] for c in range(NCORES)])

    # final selection: sort 48 by (d2 asc, j asc), take K, mask by cutoff
    order = np.lexsort((cand_j, d2c), axis=-1)[:, :K]
    d2s = np.take_along_axis(d2c, order, axis=1)
    js = np.take_along_axis(cand_j, order, axis=1)
    dds = np.take_along_axis(ddc, order, axis=1)
    vxs = np.take_along_axis(vxc, order, axis=1)
    vys = np.take_along_axis(vyc, order, axis=1)
    vzs = np.take_along_axis(vzc, order, axis=1)
    valid = d2s <= CUTOFF2
    rows_orig = prep["zord"]
    ii = rows_orig[:, None]
    src_s = np.where(valid, js, ii)
    w_s = np.where(valid, np.sqrt(dds, dtype=np.float32), np.float32(0.0))
    vx_s = np.where(valid, vxs, np.float32(0.0))
    vy_s = np.where(valid, vys, np.float32(0.0))
    vz_s = np.where(valid, vzs, np.float32(0.0))

    inv = np.empty(N, dtype=np.int64); inv[rows_orig] = np.arange(N)
    src = src_s[inv].reshape(-1).astype(np.int32)
    dst = np.repeat(np.arange(N, dtype=np.int32), K)
    w = w_s[inv].reshape(-1)
    vec = np.stack([vx_s[inv].reshape(-1), vy_s[inv].reshape(-1), vz_s[inv].reshape(-1)], axis=1)

    ar = np.arange(N, dtype=np.int32)
    edge_index = np.stack([np.concatenate([src, ar]), np.concatenate([dst, ar])])
    edge_weight = np.concatenate([w, np.zeros(N, np.float32)])
    edge_vec = np.concatenate([vec, np.zeros((N, 3), np.float32)], axis=0)
    return edge_index, edge_weight, edge_vec


# revision 9
# speedup vs baseline: 1.5853x; 1.1879x over previous
"""Deterministic radius-graph KNN (N=16384, K=32, cutoff 5.0) on 8 trn2 NeuronCores.

Strategy (data-parallel over query atoms, z-slab sharded):
  Host: z-sort atoms; per 128-query tile build a 3584-wide z-window of candidates.
  Phase A (device): -d2 approx via PE fp32 matmul (K=5 fused: 2*dot - sq_i - sq_j)
    on box-centered coords; ACT writes the fp16-cast values into the high
    halfword of an int32 tile whose low halfword GPSIMD pre-fills with the
    window column id (the composed word compares as f32 in fp16 order with
    the column as tiebreak), then hierarchical top-48 selection per query
    (14 strided chunks x cap-16 via max8/match_replace only -- no max_index,
    no keying passes; indices decode from the low halfword).
  Host: decode candidate indices from key bits, gather candidate fields.
  Phase B (device): bit-exact XLA:CPU d2/dd/vec for the 47 best candidates per
    query, via Dekker-split + TwoSum fp32 emulation of the CPU's fma-chain dot
    on DVE, with vec/dd offloaded to GPSIMD in parallel.
  Host: final sort of 47 by (d2, j), cutoff mask, self-pair padding, assembly.

The reference's top-k ordering is knife-edge sensitive to d2 rounding, so phase B
reproduces XLA:CPU arithmetic bit-for-bit; the final output matches the oracle
bitwise on uniform inputs.
"""
import numpy as np

import concourse.bacc as bacc
import concourse.mybir as mybir
import concourse.tile as tile
from concourse.bass_utils import run_bass_kernel_spmd

# ---------------- constants (hardcoded for the given problem) ----------------
N = 16384
K = 32
CUTOFF2 = np.float32(25.0)
NCORES = 8
RPC = N // NCORES          # queries per core
TPC = RPC // 128           # row-tiles per core
NTILES = N // 128
W = 3584                   # candidate window (= 7*512 = 14*256)
S = 14                     # strided chunks per window
CAP = 16                   # survivors per chunk
NCAND = S * CAP            # 224
NSEL = 48                  # stage-2 extracted (incl. self)
NKEEP = 47                 # candidates refined in phase B
SENT = -3.0e38
FREE = TPC * NKEEP
NF = 20

F32 = mybir.dt.float32
U32 = mybir.dt.uint32
I32 = mybir.dt.int32
F16 = mybir.dt.float16
OP = mybir.AluOpType

_CACHE = {}


# ---------------- device programs ----------------
def _build_phase_a():
    nc = bacc.Bacc("TRN2", target_bir_lowering=False, debug=False,
                   enable_asserts=True, num_devices=NCORES)
    ab = nc.dram_tensor("ab", [TPC, 5, 128 + W], F32, kind="ExternalInput")
    k48 = nc.dram_tensor("k48", [TPC, 128, NSEL], F32, kind="ExternalOutput")

    with tile.TileContext(nc) as tc:
        with tc.tile_pool(name="inp", bufs=3) as inp, \
             tc.tile_pool(name="big", bufs=2) as big, \
             tc.tile_pool(name="cand", bufs=2) as cp, \
             tc.tile_pool(name="out", bufs=2) as op_, \
             tc.tile_pool(name="psum", bufs=4, space="PSUM") as pp:
            for t in range(TPC):
                tab = inp.tile([5, 128 + W], F32, tag="tab", name="tab")
                nc.sync.dma_start(out=tab, in_=ab[t])
                # Key tile: GPSIMD pre-fills the low halfword of each int32 with
                # the window column id; ACT then writes the fp16-cast matmul
                # result into the high halfword. The composed word compares (as
                # f32) exactly like the fp16 value with the column as tiebreak,
                # so selection needs no separate keying passes on DVE.
                kb = big.tile([128, W], I32, tag="kb", name="kb")
                nc.gpsimd.iota(kb, pattern=[[1, W]], base=0, channel_multiplier=0)
                kh = kb.bitcast(F16)
                for b in range(W // 512):
                    ps = pp.tile([128, 512], F32, tag="ps", name="ps")
                    nc.tensor.matmul(ps, tab[:, :128],
                                     tab[:, 128 + b * 512:128 + (b + 1) * 512],
                                     start=True, stop=True)
                    nc.scalar.activation(kh[:, 2 * b * 512 + 1:2 * (b + 1) * 512:2],
                                         ps, mybir.ActivationFunctionType.Copy)
                neg = kb.bitcast(F32)
                cand = cp.tile([128, NCAND], F32, tag="cand", name="cand")
                for c in range(S):
                    view = neg[:, c:W:S]
                    v0 = cand[:, c * CAP:c * CAP + 8]
                    nc.vector.max(out=v0, in_=view)
                    nc.vector.match_replace(out=view, in_to_replace=v0,
                                            in_values=view, imm_value=SENT)
                    nc.vector.max(out=cand[:, c * CAP + 8:c * CAP + 16], in_=view)
                q = op_.tile([128, NSEL], F32, tag="q", name="q")
                for r in range(NSEL // 8):
                    nc.vector.max(out=q[:, r * 8:(r + 1) * 8], in_=cand)
                    if r < NSEL // 8 - 1:
                        nc.vector.match_replace(out=cand,
                                                in_to_replace=q[:, r * 8:(r + 1) * 8],
                                                in_values=cand, imm_value=SENT)
                nc.sync.dma_start(out=k48[t], in_=q)
    nc.compile()
    return nc


def _build_phase_b():
    nc = bacc.Bacc("TRN2", target_bir_lowering=False, debug=False,
                   enable_asserts=True, num_devices=NCORES)
    inp = nc.dram_tensor("inp", [NF, 128, FREE], F32, kind="ExternalInput")
    out = nc.dram_tensor("out", [5, 128, FREE], F32, kind="ExternalOutput")

    with tile.TileContext(nc) as tc:
        with tc.tile_pool(name="f", bufs=1) as fp, \
             tc.tile_pool(name="s", bufs=1) as sp:
            f = [fp.tile([128, FREE], F32, tag=f"f{i}", name=f"f{i}") for i in range(NF)]
            for i in range(NF):
                nc.sync.dma_start(out=f[i], in_=inp[i])
            (ax, ay, az, axh, axl, ayh, ayl, azh, azl, sqa,
             bx, by, bz, bxh, bxl, byh, byl, bzh, bzl, sqb) = f

            def T(tag):
                return sp.tile([128, FREE], F32, tag=tag, name=tag)

            def tt(o, a, b, op):
                nc.vector.tensor_tensor(out=o, in0=a, in1=b, op=op)

            def fma(a, b, ah, al, bh, bl, c, outt):
                # outt = round(a*b + c), bit-exact (Dekker product + TwoSum)
                ph = T("ph"); tt(ph, a, b, OP.mult)
                w = T("w"); e = T("e")
                tt(w, ah, bh, OP.mult)
                tt(e, w, ph, OP.subtract)
                tt(w, ah, bl, OP.mult)
                tt(e, e, w, OP.add)
                tt(w, al, bh, OP.mult)
                tt(e, e, w, OP.add)
                tt(w, al, bl, OP.mult)
                tt(e, e, w, OP.add)           # e = exact residual of a*b
                s = T("s"); tt(s, ph, c, OP.add)
                bv = T("bv"); tt(bv, s, ph, OP.subtract)
                ta_ = T("ta_"); tt(ta_, c, bv, OP.subtract)
                av = T("av"); tt(av, s, bv, OP.subtract)
                tb_ = T("tb_"); tt(tb_, ph, av, OP.subtract)
                tt(ta_, ta_, tb_, OP.add)     # exact residual of ph + c
                tt(ta_, ta_, e, OP.add)
                tt(outt, s, ta_, OP.add)

            m0 = T("m0"); tt(m0, ax, bx, OP.mult)
            f1 = T("f1"); fma(ay, by, ayh, ayl, byh, byl, m0, f1)
            dot = T("dot"); fma(az, bz, azh, azl, bzh, bzl, f1, dot)

            d2 = sp.tile([128, FREE], F32, tag="d2", name="d2")
            tt(d2, sqa, sqb, OP.add)
            nc.vector.tensor_scalar(dot, dot, 2.0, scalar2=None, op0=OP.mult)
            tt(d2, d2, dot, OP.subtract)
            nc.vector.tensor_scalar(d2, d2, 0.0, scalar2=None, op0=OP.max)

            def gt(o, a, b, op):
                nc.gpsimd.tensor_tensor(out=o, in0=a, in1=b, op=op)

            vx = sp.tile([128, FREE], F32, tag="vx", name="vx"); gt(vx, bx, ax, OP.subtract)
            vy = sp.tile([128, FREE], F32, tag="vy", name="vy"); gt(vy, by, ay, OP.subtract)
            vz = sp.tile([128, FREE], F32, tag="vz", name="vz"); gt(vz, bz, az, OP.subtract)
            dd = sp.tile([128, FREE], F32, tag="dd", name="dd")
            t1 = sp.tile([128, FREE], F32, tag="sq1", name="sq1"); gt(t1, vx, vx, OP.mult)
            t2 = sp.tile([128, FREE], F32, tag="sq2", name="sq2"); gt(t2, vy, vy, OP.mult)
            gt(dd, t1, t2, OP.add)
            gt(t1, vz, vz, OP.mult)
            gt(dd, dd, t1, OP.add)

            for i, tl in enumerate([d2, dd, vx, vy, vz]):
                nc.sync.dma_start(out=out[i], in_=tl)
    nc.compile()
    return nc


# ---------------- host glue ----------------
def _host_sq(pos):
    x, y, z = pos[:, 0], pos[:, 1], pos[:, 2]
    return (x * x + y * y) + z * z


def _host_prep(pos):
    sq = _host_sq(pos)
    zord = np.argsort(pos[:, 2], kind="stable").astype(np.int64)
    posz = pos[zord]
    # Phase A works on box-centered coords: same exact-math d2, ~4x smaller
    # magnitudes -> ~4x smaller PE fp32 rounding noise -> larger rank margins.
    posc = posz - np.float32(28.0)
    sqc = _host_sq(posc)
    zs = posz[:, 2].astype(np.float64)
    lo = np.zeros(NTILES, dtype=np.int64)
    for t in range(NTILES):
        l = np.searchsorted(zs, zs[t * 128] - 5.0, side="left")
        h = np.searchsorted(zs, zs[t * 128 + 127] + 5.0, side="right")
        if h - l > W:
            l = max(0, (l + h - W) // 2)   # best effort (should not happen)
        lo[t] = min(l, N - W)
    ab = np.zeros((NTILES, 5, 128 + W), dtype=np.float32)
    for t in range(NTILES):
        q = posc[t * 128:(t + 1) * 128]
        ab[t, 0, :128] = np.float32(2.0) * q[:, 0]
        ab[t, 1, :128] = np.float32(2.0) * q[:, 1]
        ab[t, 2, :128] = np.float32(2.0) * q[:, 2]
        ab[t, 3, :128] = 1.0
        ab[t, 4, :128] = -sqc[t * 128:(t + 1) * 128]
        w = posc[lo[t]:lo[t] + W]
        ab[t, 0, 128:] = w[:, 0]
        ab[t, 1, 128:] = w[:, 1]
        ab[t, 2, 128:] = w[:, 2]
        ab[t, 3, 128:] = -sqc[lo[t]:lo[t] + W]
        ab[t, 4, 128:] = 1.0
    return dict(pos=pos, sq=sq, zord=zord, lo=lo, ab=ab)


def _decode_phase_a(prep, k48_all):
    """k48_all: [NTILES,128,NSEL] keyed f32 -> cand_j [N, NKEEP] original indices."""
    lo = prep["lo"]; zord = prep["zord"]
    bits = k48_all.view(np.uint32)
    isneg = (bits >> 31) == 1                     # real candidates (self is +0-keyed)
    wincol = (bits & np.uint32(0xFFFF)).astype(np.int64)
    cand_j = np.empty((N, NKEEP), dtype=np.int64)
    for t in range(NTILES):
        gs = lo[t] + wincol[t]
        selfidx = (t * 128 + np.arange(128))[:, None]
        ok = isneg[t] & (gs != selfidx)
        for r in range(128):
            g = gs[r][ok[r]]
            _, first = np.unique(g, return_index=True)
            g = g[np.sort(first)]
            if len(g) < NKEEP:       # should not happen; pad defensively
                pool = [v for v in range(lo[t], lo[t] + W)
                        if v not in set(g) and v != t * 128 + r][:NKEEP - len(g)]
                g = np.concatenate([g, np.array(pool, dtype=np.int64)])
            cand_j[t * 128 + r] = zord[g[:NKEEP]]
    return cand_j


def _split(v):
    c = np.float32(4097.0)
    t = np.float32(c * v)
    hi = np.float32(t - np.float32(t - v))
    return hi, np.float32(v - hi)


def _phase_b_inputs(prep, cand_j, core):
    pos = prep["pos"]; sq = prep["sq"]; zord = prep["zord"]
    rows_orig = zord[np.arange(core * RPC, (core + 1) * RPC)]
    cj = cand_j[core * RPC:(core + 1) * RPC]
    a = pos[rows_orig]
    b = pos[cj]
    arr = np.empty((NF, 128, FREE), dtype=np.float32)

    def fill(fi, vals):
        arr[fi] = vals.reshape(TPC, 128, NKEEP).transpose(1, 0, 2).reshape(128, FREE)

    for d in range(3):
        h, l = _split(a[:, d])
        fill(d, np.broadcast_to(a[:, d][:, None], (RPC, NKEEP)))
        fill(3 + 2 * d, np.broadcast_to(h[:, None], (RPC, NKEEP)))
        fill(4 + 2 * d, np.broadcast_to(l[:, None], (RPC, NKEEP)))
        h, l = _split(b[:, :, d])
        fill(10 + d, b[:, :, d])
        fill(13 + 2 * d, h)
        fill(14 + 2 * d, l)
    fill(9, np.broadcast_to(sq[rows_orig][:, None], (RPC, NKEEP)))
    fill(19, sq[cj])
    return arr


def _decode_phase_b(outs):
    full = []
    for fi in range(5):
        per = [outs[c][fi].reshape(128, TPC, NKEEP).transpose(1, 0, 2).reshape(RPC, NKEEP)
               for c in range(NCORES)]
        full.append(np.concatenate(per, axis=0))
    return full


# ---------------- entry point ----------------
def kernel(pos):
    pos = np.ascontiguousarray(np.asarray(pos, dtype=np.float32))
    assert pos.shape == (N, 3)
    prep = _host_prep(pos)

    if "a" not in _CACHE:
        _CACHE["a"] = _build_phase_a()
    nca = _CACHE["a"]
    in_a = [{"ab": np.ascontiguousarray(prep["ab"][c * TPC:(c + 1) * TPC])}
            for c in range(NCORES)]
    res_a = run_bass_kernel_spmd(nca, in_a, list(range(NCORES)))
    k48 = np.stack([res_a.results[c]["k48"] for c in range(NCORES)]).reshape(NTILES, 128, NSEL)

    cand_j = _decode_phase_a(prep, k48)

    if "b" not in _CACHE:
        _CACHE["b"] = _build_phase_b()
    ncb = _CACHE["b"]
    in_b = [{"inp": _phase_b_inputs(prep, cand_j, c)} for c in range(NCORES)]
    res_b = run_bass_kernel_spmd(ncb, in_b, list(range(NCORES)))
    d2c, ddc, vxc, vyc, vzc = _decode_phase_b([res_b.results[c]["out"] for c in range(NCORES)])

    # final selection: sort 48 by (d2 asc, j asc), take K, mask by cutoff
    order = np.lexsort((cand_j, d2c), axis=-1)[:, :K]
    d2s = np.take_along_axis(d2c, order, axis=1)
    js = np.take_along_axis(cand_j, order, axis=1)
    dds = np.take_along_axis(ddc, order, axis=1)
    vxs = np.take_along_axis(vxc, order, axis=1)
    vys = np.take_along_axis(vyc, order, axis=1)
    vzs = np.take_along_axis(vzc, order, axis=1)
    valid = d2s <= CUTOFF2
    rows_orig = prep["zord"]
    ii = rows_orig[:, None]
    src_s = np.where(valid, js, ii)
    w_s = np.where(valid, np.sqrt(dds, dtype=np.float32), np.float32(0.0))
    vx_s = np.where(valid, vxs, np.float32(0.0))
    vy_s = np.where(valid, vys, np.float32(0.0))
    vz_s = np.where(valid, vzs, np.float32(0.0))

    inv = np.empty(N, dtype=np.int64); inv[rows_orig] = np.arange(N)
    src = src_s[inv].reshape(-1).astype(np.int32)
    dst = np.repeat(np.arange(N, dtype=np.int32), K)
    w = w_s[inv].reshape(-1)
    vec = np.stack([vx_s[inv].reshape(-1), vy_s[inv].reshape(-1), vz_s[inv].reshape(-1)], axis=1)

    ar = np.arange(N, dtype=np.int32)
    edge_index = np.stack([np.concatenate([src, ar]), np.concatenate([dst, ar])])
    edge_weight = np.concatenate([w, np.zeros(N, np.float32)])
    edge_vec = np.concatenate([vec, np.zeros((N, 3), np.float32)], axis=0)
    return edge_index, edge_weight, edge_vec


# revision 10
# speedup vs baseline: 1.6680x; 1.0521x over previous
"""Deterministic radius-graph KNN (N=16384, K=32, cutoff 5.0) on 8 trn2 NeuronCores.

Strategy (data-parallel over query atoms, z-slab sharded):
  Host: z-sort atoms; per 128-query tile build a 3584-wide z-window of candidates.
  Phase A (device): -d2 approx via PE fp32 matmul (K=5 fused: 2*dot - sq_i - sq_j)
    on box-centered coords; ACT writes the fp16-cast values into the high
    halfword of an int32 tile whose low halfword GPSIMD pre-fills with the
    window column id (the composed word compares as f32 in fp16 order with
    the column as tiebreak), then hierarchical top-48 selection per query
    (14 strided chunks x cap-16 via max8/match_replace only, then top-40 of
    224 -- no max_index, no keying passes; indices decode from the low halfword).
  Host: decode candidate indices from key bits, gather candidate fields.
  Phase B (device): bit-exact XLA:CPU d2/dd/vec for the 39 best candidates per
    query, via Dekker-split + TwoSum fp32 emulation of the CPU's fma-chain dot
    on DVE, with vec/dd offloaded to GPSIMD in parallel.
  Host: final sort of 39 by (d2, j), cutoff mask, self-pair padding, assembly.

The reference's top-k ordering is knife-edge sensitive to d2 rounding, so phase B
reproduces XLA:CPU arithmetic bit-for-bit; the final output matches the oracle
bitwise on uniform inputs.
"""
import numpy as np

import concourse.bacc as bacc
import concourse.mybir as mybir
import concourse.tile as tile
from concourse.bass_utils import run_bass_kernel_spmd

# ---------------- constants (hardcoded for the given problem) ----------------
N = 16384
K = 32
CUTOFF2 = np.float32(25.0)
NCORES = 8
RPC = N // NCORES          # queries per core
TPC = RPC // 128           # row-tiles per core
NTILES = N // 128
W = 3584                   # candidate window (= 7*512 = 14*256)
S = 14                     # strided chunks per window
CAP = 16                   # survivors per chunk
NCAND = S * CAP            # 224
NSEL = 40                  # stage-2 extracted (incl. self)
NKEEP = 39                 # candidates refined in phase B
SENT = -3.0e38
FREE = TPC * NKEEP
NF = 20

F32 = mybir.dt.float32
U32 = mybir.dt.uint32
I32 = mybir.dt.int32
F16 = mybir.dt.float16
OP = mybir.AluOpType

_CACHE = {}


# ---------------- device programs ----------------
def _build_phase_a():
    nc = bacc.Bacc("TRN2", target_bir_lowering=False, debug=False,
                   enable_asserts=True, num_devices=NCORES)
    ab = nc.dram_tensor("ab", [TPC, 5, 128 + W], F32, kind="ExternalInput")
    k48 = nc.dram_tensor("k48", [TPC, 128, NSEL], F32, kind="ExternalOutput")

    with tile.TileContext(nc) as tc:
        with tc.tile_pool(name="inp", bufs=3) as inp, \
             tc.tile_pool(name="big", bufs=2) as big, \
             tc.tile_pool(name="cand", bufs=2) as cp, \
             tc.tile_pool(name="out", bufs=2) as op_, \
             tc.tile_pool(name="psum", bufs=4, space="PSUM") as pp:
            for t in range(TPC):
                tab = inp.tile([5, 128 + W], F32, tag="tab", name="tab")
                nc.sync.dma_start(out=tab, in_=ab[t])
                # Key tile: GPSIMD pre-fills the low halfword of each int32 with
                # the window column id; ACT then writes the fp16-cast matmul
                # result into the high halfword. The composed word compares (as
                # f32) exactly like the fp16 value with the column as tiebreak,
                # so selection needs no separate keying passes on DVE.
                kb = big.tile([128, W], I32, tag="kb", name="kb")
                nc.gpsimd.iota(kb, pattern=[[1, W]], base=0, channel_multiplier=0)
                kh = kb.bitcast(F16)
                for b in range(W // 512):
                    ps = pp.tile([128, 512], F32, tag="ps", name="ps")
                    nc.tensor.matmul(ps, tab[:, :128],
                                     tab[:, 128 + b * 512:128 + (b + 1) * 512],
                                     start=True, stop=True)
                    nc.scalar.activation(kh[:, 2 * b * 512 + 1:2 * (b + 1) * 512:2],
                                         ps, mybir.ActivationFunctionType.Copy)
                neg = kb.bitcast(F32)
                cand = cp.tile([128, NCAND], F32, tag="cand", name="cand")
                for c in range(S):
                    view = neg[:, c:W:S]
                    v0 = cand[:, c * CAP:c * CAP + 8]
                    nc.vector.max(out=v0, in_=view)
                    nc.vector.match_replace(out=view, in_to_replace=v0,
                                            in_values=view, imm_value=SENT)
                    nc.vector.max(out=cand[:, c * CAP + 8:c * CAP + 16], in_=view)
                q = op_.tile([128, NSEL], F32, tag="q", name="q")
                for r in range(NSEL // 8):
                    nc.vector.max(out=q[:, r * 8:(r + 1) * 8], in_=cand)
                    if r < NSEL // 8 - 1:
                        nc.vector.match_replace(out=cand,
                                                in_to_replace=q[:, r * 8:(r + 1) * 8],
                                                in_values=cand, imm_value=SENT)
                nc.sync.dma_start(out=k48[t], in_=q)
    nc.compile()
    return nc


def _build_phase_b():
    nc = bacc.Bacc("TRN2", target_bir_lowering=False, debug=False,
                   enable_asserts=True, num_devices=NCORES)
    inp = nc.dram_tensor("inp", [NF, 128, FREE], F32, kind="ExternalInput")
    out = nc.dram_tensor("out", [5, 128, FREE], F32, kind="ExternalOutput")

    with tile.TileContext(nc) as tc:
        with tc.tile_pool(name="f", bufs=1) as fp, \
             tc.tile_pool(name="s", bufs=1) as sp:
            f = [fp.tile([128, FREE], F32, tag=f"f{i}", name=f"f{i}") for i in range(NF)]
            for i in range(NF):
                nc.sync.dma_start(out=f[i], in_=inp[i])
            (ax, ay, az, axh, axl, ayh, ayl, azh, azl, sqa,
             bx, by, bz, bxh, bxl, byh, byl, bzh, bzl, sqb) = f

            def T(tag):
                return sp.tile([128, FREE], F32, tag=tag, name=tag)

            def tt(o, a, b, op):
                nc.vector.tensor_tensor(out=o, in0=a, in1=b, op=op)

            def fma(a, b, ah, al, bh, bl, c, outt):
                # outt = round(a*b + c), bit-exact (Dekker product + TwoSum)
                ph = T("ph"); tt(ph, a, b, OP.mult)
                w = T("w"); e = T("e")
                tt(w, ah, bh, OP.mult)
                tt(e, w, ph, OP.subtract)
                tt(w, ah, bl, OP.mult)
                tt(e, e, w, OP.add)
                tt(w, al, bh, OP.mult)
                tt(e, e, w, OP.add)
                tt(w, al, bl, OP.mult)
                tt(e, e, w, OP.add)           # e = exact residual of a*b
                s = T("s"); tt(s, ph, c, OP.add)
                bv = T("bv"); tt(bv, s, ph, OP.subtract)
                ta_ = T("ta_"); tt(ta_, c, bv, OP.subtract)
                av = T("av"); tt(av, s, bv, OP.subtract)
                tb_ = T("tb_"); tt(tb_, ph, av, OP.subtract)
                tt(ta_, ta_, tb_, OP.add)     # exact residual of ph + c
                tt(ta_, ta_, e, OP.add)
                tt(outt, s, ta_, OP.add)

            m0 = T("m0"); tt(m0, ax, bx, OP.mult)
            f1 = T("f1"); fma(ay, by, ayh, ayl, byh, byl, m0, f1)
            dot = T("dot"); fma(az, bz, azh, azl, bzh, bzl, f1, dot)

            d2 = sp.tile([128, FREE], F32, tag="d2", name="d2")
            tt(d2, sqa, sqb, OP.add)
            nc.vector.tensor_scalar(dot, dot, 2.0, scalar2=None, op0=OP.mult)
            tt(d2, d2, dot, OP.subtract)
            nc.vector.tensor_scalar(d2, d2, 0.0, scalar2=None, op0=OP.max)

            def gt(o, a, b, op):
                nc.gpsimd.tensor_tensor(out=o, in0=a, in1=b, op=op)

            vx = sp.tile([128, FREE], F32, tag="vx", name="vx"); gt(vx, bx, ax, OP.subtract)
            vy = sp.tile([128, FREE], F32, tag="vy", name="vy"); gt(vy, by, ay, OP.subtract)
            vz = sp.tile([128, FREE], F32, tag="vz", name="vz"); gt(vz, bz, az, OP.subtract)
            dd = sp.tile([128, FREE], F32, tag="dd", name="dd")
            t1 = sp.tile([128, FREE], F32, tag="sq1", name="sq1"); gt(t1, vx, vx, OP.mult)
            t2 = sp.tile([128, FREE], F32, tag="sq2", name="sq2"); gt(t2, vy, vy, OP.mult)
            gt(dd, t1, t2, OP.add)
            gt(t1, vz, vz, OP.mult)
            gt(dd, dd, t1, OP.add)

            for i, tl in enumerate([d2, dd, vx, vy, vz]):
                nc.sync.dma_start(out=out[i], in_=tl)
    nc.compile()
    return nc


# ---------------- host glue ----------------
def _host_sq(pos):
    x, y, z = pos[:, 0], pos[:, 1], pos[:, 2]
    return (x * x + y * y) + z * z


def _host_prep(pos):
    sq = _host_sq(pos)
    zord = np.argsort(pos[:, 2], kind="stable").astype(np.int64)
    posz = pos[zord]
    # Phase A works on box-centered coords: same exact-math d2, ~4x smaller
    # magnitudes -> ~4x smaller PE fp32 rounding noise -> larger rank margins.
    posc = posz - np.float32(28.0)
    sqc = _host_sq(posc)
    zs = posz[:, 2].astype(np.float64)
    lo = np.zeros(NTILES, dtype=np.int64)
    for t in range(NTILES):
        l = np.searchsorted(zs, zs[t * 128] - 5.0, side="left")
        h = np.searchsorted(zs, zs[t * 128 + 127] + 5.0, side="right")
        if h - l > W:
            l = max(0, (l + h - W) // 2)   # best effort (should not happen)
        lo[t] = min(l, N - W)
    ab = np.zeros((NTILES, 5, 128 + W), dtype=np.float32)
    for t in range(NTILES):
        q = posc[t * 128:(t + 1) * 128]
        ab[t, 0, :128] = np.float32(2.0) * q[:, 0]
        ab[t, 1, :128] = np.float32(2.0) * q[:, 1]
        ab[t, 2, :128] = np.float32(2.0) * q[:, 2]
        ab[t, 3, :128] = 1.0
        ab[t, 4, :128] = -sqc[t * 128:(t + 1) * 128]
        w = posc[lo[t]:lo[t] + W]
        ab[t, 0, 128:] = w[:, 0]
        ab[t, 1, 128:] = w[:, 1]
        ab[t, 2, 128:] = w[:, 2]
        ab[t, 3, 128:] = -sqc[lo[t]:lo[t] + W]
        ab[t, 4, 128:] = 1.0
    return dict(pos=pos, sq=sq, zord=zord, lo=lo, ab=ab)


def _decode_phase_a(prep, k48_all):
    """k48_all: [NTILES,128,NSEL] keyed f32 -> cand_j [N, NKEEP] original indices."""
    lo = prep["lo"]; zord = prep["zord"]
    bits = k48_all.view(np.uint32)
    isneg = (bits >> 31) == 1                     # real candidates (self is +0-keyed)
    wincol = (bits & np.uint32(0xFFFF)).astype(np.int64)
    cand_j = np.empty((N, NKEEP), dtype=np.int64)
    for t in range(NTILES):
        gs = lo[t] + wincol[t]
        selfidx = (t * 128 + np.arange(128))[:, None]
        ok = isneg[t] & (gs != selfidx)
        for r in range(128):
            g = gs[r][ok[r]]
            _, first = np.unique(g, return_index=True)
            g = g[np.sort(first)]
            if len(g) < NKEEP:       # should not happen; pad defensively
                pool = [v for v in range(lo[t], lo[t] + W)
                        if v not in set(g) and v != t * 128 + r][:NKEEP - len(g)]
                g = np.concatenate([g, np.array(pool, dtype=np.int64)])
            cand_j[t * 128 + r] = zord[g[:NKEEP]]
    return cand_j


def _split(v):
    c = np.float32(4097.0)
    t = np.float32(c * v)
    hi = np.float32(t - np.float32(t - v))
    return hi, np.float32(v - hi)


def _phase_b_inputs(prep, cand_j, core):
    pos = prep["pos"]; sq = prep["sq"]; zord = prep["zord"]
    rows_orig = zord[np.arange(core * RPC, (core + 1) * RPC)]
    cj = cand_j[core * RPC:(core + 1) * RPC]
    a = pos[rows_orig]
    b = pos[cj]
    arr = np.empty((NF, 128, FREE), dtype=np.float32)

    def fill(fi, vals):
        arr[fi] = vals.reshape(TPC, 128, NKEEP).transpose(1, 0, 2).reshape(128, FREE)

    for d in range(3):
        h, l = _split(a[:, d])
        fill(d, np.broadcast_to(a[:, d][:, None], (RPC, NKEEP)))
        fill(3 + 2 * d, np.broadcast_to(h[:, None], (RPC, NKEEP)))
        fill(4 + 2 * d, np.broadcast_to(l[:, None], (RPC, NKEEP)))
        h, l = _split(b[:, :, d])
        fill(10 + d, b[:, :, d])
        fill(13 + 2 * d, h)
        fill(14 + 2 * d, l)
    fill(9, np.broadcast_to(sq[rows_orig][:, None], (RPC, NKEEP)))
    fill(19, sq[cj])
    return arr


def _decode_phase_b(outs):
    full = []
    for fi in range(5):
        per = [outs[c][fi].reshape(128, TPC, NKEEP).transpose(1, 0, 2).reshape(RPC, NKEEP)
               for c in range(NCORES)]
        full.append(np.concatenate(per, axis=0))
    return full


# ---------------- entry point ----------------
def kernel(pos):
    pos = np.ascontiguousarray(np.asarray(pos, dtype=np.float32))
    assert pos.shape == (N, 3)
    prep = _host_prep(pos)

    if "a" not in _CACHE:
        _CACHE["a"] = _build_phase_a()
    nca = _CACHE["a"]
    in_a = [{"ab": np.ascontiguousarray(prep["ab"][c * TPC:(c + 1) * TPC])}
            for c in range(NCORES)]
    res_a = run_bass_kernel_spmd(nca, in_a, list(range(NCORES)))
    k48 = np.stack([res_a.results[c]["k48"] for c in range(NCORES)]).reshape(NTILES, 128, NSEL)

    cand_j = _decode_phase_a(prep, k48)

    if "b" not in _CACHE:
        _CACHE["b"] = _build_phase_b()
    ncb = _CACHE["b"]
    in_b = [{"inp": _phase_b_inputs(prep, cand_j, c)} for c in range(NCORES)]
    res_b = run_bass_kernel_spmd(ncb, in_b, list(range(NCORES)))
    d2c, ddc, vxc, vyc, vzc = _decode_phase_b([res_b.results[c]["out"] for c in range(NCORES)])

    # final selection: sort 48 by (d2 asc, j asc), take K, mask by cutoff
    order = np.lexsort((cand_j, d2c), axis=-1)[:, :K]
    d2s = np.take_along_axis(d2c, order, axis=1)
    js = np.take_along_axis(cand_j, order, axis=1)
    dds = np.take_along_axis(ddc, order, axis=1)
    vxs = np.take_along_axis(vxc, order, axis=1)
    vys = np.take_along_axis(vyc, order, axis=1)
    vzs = np.take_along_axis(vzc, order, axis=1)
    valid = d2s <= CUTOFF2
    rows_orig = prep["zord"]
    ii = rows_orig[:, None]
    src_s = np.where(valid, js, ii)
    w_s = np.where(valid, np.sqrt(dds, dtype=np.float32), np.float32(0.0))
    vx_s = np.where(valid, vxs, np.float32(0.0))
    vy_s = np.where(valid, vys, np.float32(0.0))
    vz_s = np.where(valid, vzs, np.float32(0.0))

    inv = np.empty(N, dtype=np.int64); inv[rows_orig] = np.arange(N)
    src = src_s[inv].reshape(-1).astype(np.int32)
    dst = np.repeat(np.arange(N, dtype=np.int32), K)
    w = w_s[inv].reshape(-1)
    vec = np.stack([vx_s[inv].reshape(-1), vy_s[inv].reshape(-1), vz_s[inv].reshape(-1)], axis=1)

    ar = np.arange(N, dtype=np.int32)
    edge_index = np.stack([np.concatenate([src, ar]), np.concatenate([dst, ar])])
    edge_weight = np.concatenate([w, np.zeros(N, np.float32)])
    edge_vec = np.concatenate([vec, np.zeros((N, 3), np.float32)], axis=0)
    return edge_index, edge_weight, edge_vec


# revision 11
# speedup vs baseline: 1.7052x; 1.0223x over previous
"""Deterministic radius-graph KNN (N=16384, K=32, cutoff 5.0) on 8 trn2 NeuronCores.

Strategy (data-parallel over query atoms, z-slab sharded):
  Host: z-sort atoms; per 128-query tile build a 3584-wide z-window of candidates.
  Phase A (device): -d2 approx via PE fp32 matmul (K=5 fused: 2*dot - sq_i - sq_j)
    on box-centered coords; ACT writes the fp16-cast values into the high
    halfword of an int32 tile whose low halfword GPSIMD pre-fills with the
    window column id (the composed word compares as f32 in fp16 order with
    the column as tiebreak), then hierarchical top-48 selection per query
    (14 strided chunks x cap-16 via max8/match_replace only, then top-40 of
    224 -- no max_index, no keying passes; indices decode from the low halfword).
  Host: decode candidate indices from key bits, gather candidate fields.
  Phase B (device): bit-exact XLA:CPU d2/dd/vec for the 39 best candidates per
    query, via Dekker-split + TwoSum fp32 emulation of the CPU's fma-chain dot
    on DVE, with vec/dd offloaded to GPSIMD in parallel.
  Host: final sort of 39 by (d2, j), cutoff mask, self-pair padding, assembly.

The reference's top-k ordering is knife-edge sensitive to d2 rounding, so phase B
reproduces XLA:CPU arithmetic bit-for-bit; the final output matches the oracle
bitwise on uniform inputs.
"""
import numpy as np

import concourse.bacc as bacc
import concourse.mybir as mybir
import concourse.tile as tile
from concourse.bass_utils import run_bass_kernel_spmd

# ---------------- constants (hardcoded for the given problem) ----------------
N = 16384
K = 32
CUTOFF2 = np.float32(25.0)
NCORES = 8
RPC = N // NCORES          # queries per core
TPC = RPC // 128           # row-tiles per core
NTILES = N // 128
W = 3584                   # candidate window (= 7*512 = 14*256)
S = 14                     # strided chunks per window
CAP = 16                   # survivors per chunk
NCAND = S * CAP            # 224
NSEL = 40                  # stage-2 extracted (incl. self)
NKEEP = 39                 # candidates refined in phase B
SENT = -3.0e38
FREE = TPC * NKEEP
NF = 16

F32 = mybir.dt.float32
U32 = mybir.dt.uint32
I32 = mybir.dt.int32
F16 = mybir.dt.float16
OP = mybir.AluOpType

_CACHE = {}


# ---------------- device programs ----------------
def _build_phase_a():
    nc = bacc.Bacc("TRN2", target_bir_lowering=False, debug=False,
                   enable_asserts=True, num_devices=NCORES)
    ab = nc.dram_tensor("ab", [TPC, 5, 128 + W], F32, kind="ExternalInput")
    k48 = nc.dram_tensor("k48", [TPC, 128, NSEL], F32, kind="ExternalOutput")

    with tile.TileContext(nc) as tc:
        with tc.tile_pool(name="inp", bufs=3) as inp, \
             tc.tile_pool(name="big", bufs=2) as big, \
             tc.tile_pool(name="cand", bufs=2) as cp, \
             tc.tile_pool(name="out", bufs=2) as op_, \
             tc.tile_pool(name="psum", bufs=4, space="PSUM") as pp:
            for t in range(TPC):
                tab = inp.tile([5, 128 + W], F32, tag="tab", name="tab")
                nc.sync.dma_start(out=tab, in_=ab[t])
                # Key tile: GPSIMD pre-fills the low halfword of each int32 with
                # the window column id; ACT then writes the fp16-cast matmul
                # result into the high halfword. The composed word compares (as
                # f32) exactly like the fp16 value with the column as tiebreak,
                # so selection needs no separate keying passes on DVE.
                kb = big.tile([128, W], I32, tag="kb", name="kb")
                nc.gpsimd.iota(kb, pattern=[[1, W]], base=0, channel_multiplier=0)
                kh = kb.bitcast(F16)
                for b in range(W // 512):
                    ps = pp.tile([128, 512], F32, tag="ps", name="ps")
                    nc.tensor.matmul(ps, tab[:, :128],
                                     tab[:, 128 + b * 512:128 + (b + 1) * 512],
                                     start=True, stop=True)
                    nc.scalar.activation(kh[:, 2 * b * 512 + 1:2 * (b + 1) * 512:2],
                                         ps, mybir.ActivationFunctionType.Copy)
                neg = kb.bitcast(F32)
                cand = cp.tile([128, NCAND], F32, tag="cand", name="cand")
                for c in range(S):
                    view = neg[:, c:W:S]
                    v0 = cand[:, c * CAP:c * CAP + 8]
                    nc.vector.max(out=v0, in_=view)
                    nc.vector.match_replace(out=view, in_to_replace=v0,
                                            in_values=view, imm_value=SENT)
                    nc.vector.max(out=cand[:, c * CAP + 8:c * CAP + 16], in_=view)
                q = op_.tile([128, NSEL], F32, tag="q", name="q")
                for r in range(NSEL // 8):
                    nc.vector.max(out=q[:, r * 8:(r + 1) * 8], in_=cand)
                    if r < NSEL // 8 - 1:
                        nc.vector.match_replace(out=cand,
                                                in_to_replace=q[:, r * 8:(r + 1) * 8],
                                                in_values=cand, imm_value=SENT)
                nc.sync.dma_start(out=k48[t], in_=q)
    nc.compile()
    return nc


def _build_phase_b():
    nc = bacc.Bacc("TRN2", target_bir_lowering=False, debug=False,
                   enable_asserts=True, num_devices=NCORES)
    inp = nc.dram_tensor("inp", [NF, 128, FREE], F32, kind="ExternalInput")
    out = nc.dram_tensor("out", [5, 128, FREE], F32, kind="ExternalOutput")

    with tile.TileContext(nc) as tc:
        with tc.tile_pool(name="f", bufs=1) as fp, \
             tc.tile_pool(name="s", bufs=1) as sp:
            f = [fp.tile([128, FREE], F32, tag=f"f{i}", name=f"f{i}") for i in range(NF)]
            # issue DMAs in first-use order so the fma chain starts early
            for i in [0, 8, 1, 9, 3, 4, 11, 12, 2, 10, 5, 6, 13, 14, 7, 15]:
                nc.sync.dma_start(out=f[i], in_=inp[i])
            (ax, ay, az, ayh, ayl, azh, azl, sqa,
             bx, by, bz, byh, byl, bzh, bzl, sqb) = f

            def T(tag):
                return sp.tile([128, FREE], F32, tag=tag, name=tag)

            def tt(o, a, b, op):
                nc.vector.tensor_tensor(out=o, in0=a, in1=b, op=op)

            def fma(a, b, ah, al, bh, bl, c, outt):
                # outt = round(a*b + c), bit-exact (Dekker product + TwoSum)
                ph = T("ph"); tt(ph, a, b, OP.mult)
                w = T("w"); e = T("e")
                tt(w, ah, bh, OP.mult)
                tt(e, w, ph, OP.subtract)
                tt(w, ah, bl, OP.mult)
                tt(e, e, w, OP.add)
                tt(w, al, bh, OP.mult)
                tt(e, e, w, OP.add)
                tt(w, al, bl, OP.mult)
                tt(e, e, w, OP.add)           # e = exact residual of a*b
                s = T("s"); tt(s, ph, c, OP.add)
                bv = T("bv"); tt(bv, s, ph, OP.subtract)
                ta_ = T("ta_"); tt(ta_, c, bv, OP.subtract)
                av = T("av"); tt(av, s, bv, OP.subtract)
                tb_ = T("tb_"); tt(tb_, ph, av, OP.subtract)
                tt(ta_, ta_, tb_, OP.add)     # exact residual of ph + c
                tt(ta_, ta_, e, OP.add)
                tt(outt, s, ta_, OP.add)

            m0 = T("m0"); tt(m0, ax, bx, OP.mult)
            f1 = T("f1"); fma(ay, by, ayh, ayl, byh, byl, m0, f1)
            dot = T("dot"); fma(az, bz, azh, azl, bzh, bzl, f1, dot)

            d2 = sp.tile([128, FREE], F32, tag="d2", name="d2")
            tt(d2, sqa, sqb, OP.add)
            nc.vector.tensor_scalar(dot, dot, 2.0, scalar2=None, op0=OP.mult)
            tt(d2, d2, dot, OP.subtract)
            nc.vector.tensor_scalar(d2, d2, 0.0, scalar2=None, op0=OP.max)

            def gt(o, a, b, op):
                nc.gpsimd.tensor_tensor(out=o, in0=a, in1=b, op=op)

            vx = sp.tile([128, FREE], F32, tag="vx", name="vx"); gt(vx, bx, ax, OP.subtract)
            vy = sp.tile([128, FREE], F32, tag="vy", name="vy"); gt(vy, by, ay, OP.subtract)
            vz = sp.tile([128, FREE], F32, tag="vz", name="vz"); gt(vz, bz, az, OP.subtract)
            dd = sp.tile([128, FREE], F32, tag="dd", name="dd")
            t1 = sp.tile([128, FREE], F32, tag="sq1", name="sq1"); gt(t1, vx, vx, OP.mult)
            t2 = sp.tile([128, FREE], F32, tag="sq2", name="sq2"); gt(t2, vy, vy, OP.mult)
            gt(dd, t1, t2, OP.add)
            gt(t1, vz, vz, OP.mult)
            gt(dd, dd, t1, OP.add)

            for i, tl in enumerate([d2, dd, vx, vy, vz]):
                nc.sync.dma_start(out=out[i], in_=tl)
    nc.compile()
    return nc


# ---------------- host glue ----------------
def _host_sq(pos):
    x, y, z = pos[:, 0], pos[:, 1], pos[:, 2]
    return (x * x + y * y) + z * z


def _host_prep(pos):
    sq = _host_sq(pos)
    zord = np.argsort(pos[:, 2], kind="stable").astype(np.int64)
    posz = pos[zord]
    # Phase A works on box-centered coords: same exact-math d2, ~4x smaller
    # magnitudes -> ~4x smaller PE fp32 rounding noise -> larger rank margins.
    posc = posz - np.float32(28.0)
    sqc = _host_sq(posc)
    zs = posz[:, 2].astype(np.float64)
    lo = np.zeros(NTILES, dtype=np.int64)
    for t in range(NTILES):
        l = np.searchsorted(zs, zs[t * 128] - 5.0, side="left")
        h = np.searchsorted(zs, zs[t * 128 + 127] + 5.0, side="right")
        if h - l > W:
            l = max(0, (l + h - W) // 2)   # best effort (should not happen)
        lo[t] = min(l, N - W)
    ab = np.zeros((NTILES, 5, 128 + W), dtype=np.float32)
    for t in range(NTILES):
        q = posc[t * 128:(t + 1) * 128]
        ab[t, 0, :128] = np.float32(2.0) * q[:, 0]
        ab[t, 1, :128] = np.float32(2.0) * q[:, 1]
        ab[t, 2, :128] = np.float32(2.0) * q[:, 2]
        ab[t, 3, :128] = 1.0
        ab[t, 4, :128] = -sqc[t * 128:(t + 1) * 128]
        w = posc[lo[t]:lo[t] + W]
        ab[t, 0, 128:] = w[:, 0]
        ab[t, 1, 128:] = w[:, 1]
        ab[t, 2, 128:] = w[:, 2]
        ab[t, 3, 128:] = -sqc[lo[t]:lo[t] + W]
        ab[t, 4, 128:] = 1.0
    return dict(pos=pos, sq=sq, zord=zord, lo=lo, ab=ab)


def _decode_phase_a(prep, k48_all):
    """k48_all: [NTILES,128,NSEL] keyed f32 -> cand_j [N, NKEEP] original indices."""
    lo = prep["lo"]; zord = prep["zord"]
    bits = k48_all.view(np.uint32)
    isneg = (bits >> 31) == 1                     # real candidates (self is +0-keyed)
    wincol = (bits & np.uint32(0xFFFF)).astype(np.int64)
    cand_j = np.empty((N, NKEEP), dtype=np.int64)
    for t in range(NTILES):
        gs = lo[t] + wincol[t]
        selfidx = (t * 128 + np.arange(128))[:, None]
        ok = isneg[t] & (gs != selfidx)
        for r in range(128):
            g = gs[r][ok[r]]
            _, first = np.unique(g, return_index=True)
            g = g[np.sort(first)]
            if len(g) < NKEEP:       # should not happen; pad defensively
                pool = [v for v in range(lo[t], lo[t] + W)
                        if v not in set(g) and v != t * 128 + r][:NKEEP - len(g)]
                g = np.concatenate([g, np.array(pool, dtype=np.int64)])
            cand_j[t * 128 + r] = zord[g[:NKEEP]]
    return cand_j


def _split(v):
    c = np.float32(4097.0)
    t = np.float32(c * v)
    hi = np.float32(t - np.float32(t - v))
    return hi, np.float32(v - hi)


def _phase_b_inputs(prep, cand_j, core):
    pos = prep["pos"]; sq = prep["sq"]; zord = prep["zord"]
    rows_orig = zord[np.arange(core * RPC, (core + 1) * RPC)]
    cj = cand_j[core * RPC:(core + 1) * RPC]
    a = pos[rows_orig]
    b = pos[cj]
    arr = np.empty((NF, 128, FREE), dtype=np.float32)

    def fill(fi, vals):
        arr[fi] = vals.reshape(TPC, 128, NKEEP).transpose(1, 0, 2).reshape(128, FREE)

    # fields: 0 ax 1 ay 2 az 3 ayh 4 ayl 5 azh 6 azl 7 sqa
    #         8 bx 9 by 10 bz 11 byh 12 byl 13 bzh 14 bzl 15 sqb
    for d in range(3):
        fill(d, np.broadcast_to(a[:, d][:, None], (RPC, NKEEP)))
        fill(8 + d, b[:, :, d])
    for d, (fa, fb) in enumerate([(None, None), ((3, 4), (11, 12)), ((5, 6), (13, 14))]):
        if fa is None:
            continue
        h, l = _split(a[:, d])
        fill(fa[0], np.broadcast_to(h[:, None], (RPC, NKEEP)))
        fill(fa[1], np.broadcast_to(l[:, None], (RPC, NKEEP)))
        h, l = _split(b[:, :, d])
        fill(fb[0], h)
        fill(fb[1], l)
    fill(7, np.broadcast_to(sq[rows_orig][:, None], (RPC, NKEEP)))
    fill(15, sq[cj])
    return arr


def _decode_phase_b(outs):
    full = []
    for fi in range(5):
        per = [outs[c][fi].reshape(128, TPC, NKEEP).transpose(1, 0, 2).reshape(RPC, NKEEP)
               for c in range(NCORES)]
        full.append(np.concatenate(per, axis=0))
    return full


# ---------------- entry point ----------------
def kernel(pos):
    pos = np.ascontiguousarray(np.asarray(pos, dtype=np.float32))
    assert pos.shape == (N, 3)
    prep = _host_prep(pos)

    if "a" not in _CACHE:
        _CACHE["a"] = _build_phase_a()
    nca = _CACHE["a"]
    in_a = [{"ab": np.ascontiguousarray(prep["ab"][c * TPC:(c + 1) * TPC])}
            for c in range(NCORES)]
    res_a = run_bass_kernel_spmd(nca, in_a, list(range(NCORES)))
    k48 = np.stack([res_a.results[c]["k48"] for c in range(NCORES)]).reshape(NTILES, 128, NSEL)

    cand_j = _decode_phase_a(prep, k48)

    if "b" not in _CACHE:
        _CACHE["b"] = _build_phase_b()
    ncb = _CACHE["b"]
    in_b = [{"inp": _phase_b_inputs(prep, cand_j, c)} for c in range(NCORES)]
    res_b = run_bass_kernel_spmd(ncb, in_b, list(range(NCORES)))
    d2c, ddc, vxc, vyc, vzc = _decode_phase_b([res_b.results[c]["out"] for c in range(NCORES)])

    # final selection: sort 48 by (d2 asc, j asc), take K, mask by cutoff
    order = np.lexsort((cand_j, d2c), axis=-1)[:, :K]
    d2s = np.take_along_axis(d2c, order, axis=1)
    js = np.take_along_axis(cand_j, order, axis=1)
    dds = np.take_along_axis(ddc, order, axis=1)
    vxs = np.take_along_axis(vxc, order, axis=1)
    vys = np.take_along_axis(vyc, order, axis=1)
    vzs = np.take_along_axis(vzc, order, axis=1)
    valid = d2s <= CUTOFF2
    rows_orig = prep["zord"]
    ii = rows_orig[:, None]
    src_s = np.where(valid, js, ii)
    w_s = np.where(valid, np.sqrt(dds, dtype=np.float32), np.float32(0.0))
    vx_s = np.where(valid, vxs, np.float32(0.0))
    vy_s = np.where(valid, vys, np.float32(0.0))
    vz_s = np.where(valid, vzs, np.float32(0.0))

    inv = np.empty(N, dtype=np.int64); inv[rows_orig] = np.arange(N)
    src = src_s[inv].reshape(-1).astype(np.int32)
    dst = np.repeat(np.arange(N, dtype=np.int32), K)
    w = w_s[inv].reshape(-1)
    vec = np.stack([vx_s[inv].reshape(-1), vy_s[inv].reshape(-1), vz_s[inv].reshape(-1)], axis=1)

    ar = np.arange(N, dtype=np.int32)
    edge_index = np.stack([np.concatenate([src, ar]), np.concatenate([dst, ar])])
    edge_weight = np.concatenate([w, np.zeros(N, np.float32)])
    edge_vec = np.concatenate([vec, np.zeros((N, 3), np.float32)], axis=0)
    return edge_index, edge_weight, edge_vec
